# revision 1
# baseline (speedup 1.0000x reference)
"""ConvGuidedFilter Trainium2 kernel (8 NeuronCores, batch-parallel).

Strategy:
- Shard batch 16 -> 2 samples per core. Everything per-core except BN batch
  statistics, which are AllGather'd (per-channel sum/sumsq) across the 8 cores
  so training-mode BatchNorm matches the full-batch reference exactly.
- Box filter (3x3 ones, zero-pad, /count) as two tridiagonal matmuls with the
  1/count normalization folded into the constant matrices (fp32 matmuls).
- 1x1-conv MLP as block-diagonal matmuls over a channel-major pixel layout
  (4 pixel groups packed to use 128 partitions), fp32r.
- Bilinear align_corners 128->1024 upsample of A and b as two matmuls against
  a constant [128,1024] weight matrix (stage1 fp32, stage2 fp32r).
- Fuse out = A_up * hr + b_up on the vector engine straight out of PSUM.
"""
import os
import sys

for _p in ("/opt/trn_rl_repo", "/root/.axon_site/_ro/trn_rl_repo"):
    if os.path.isdir(_p) and _p not in sys.path:
        sys.path.insert(0, _p)

from contextlib import ExitStack

import numpy as np
import concourse.bass as bass
import concourse.tile as tile
from concourse import bacc, mybir
from concourse.bass_utils import run_bass_kernel_spmd

F32 = mybir.dt.float32
F32R = mybir.dt.float32r
AX = mybir.AxisListType
ALU = mybir.AluOpType
ACTF = mybir.ActivationFunctionType

B, C, n, N = 16, 3, 128, 1024  # batch, channels, lowres, hires
N_CORES, BS = 8, 2             # cores, samples per core
G = 4                          # pixel groups (32 lowres rows each)
PF = BS * 32 * n               # packed free size per partition = 8192
PT = 512                       # pixel tile (free)
NT = PF // PT                  # 16 tiles
N_TOT = float(B * n * n)       # BN pixel count (full batch)
EPS = 1e-5
BLK = 8                        # hires row blocks per plane (1024/128)


# ---------------------------------------------------------------- host consts
def _box_mats():
    Bm = np.zeros((n, n), np.float32)
    for i in range(n):
        Bm[i, max(0, i - 1):min(n, i + 2)] = 1.0
    cnt = Bm.sum(1)
    # row-box matrix with the full interior normalization 1/(3*nrow) folded in;
    # edge *columns* get a 3/2 fixup after the column 3-tap sum.
    Mh = (Bm / (3.0 * cnt[:, None])).astype(np.float32)   # [h_out, h_in]
    return np.ascontiguousarray(Mh.T)  # MhT


def _resize_mat():
    c = np.arange(N, dtype=np.float32) * ((n - 1) / (N - 1))
    i0 = np.clip(np.floor(c).astype(np.int64), 0, n - 2)
    t = (c - i0).astype(np.float32)
    R = np.zeros((N, n), np.float32)
    R[np.arange(N), i0] = 1.0 - t
    R[np.arange(N), i0 + 1] += t
    return np.ascontiguousarray(R.T)  # RT [n_in=128, n_out=1024]


def _host_consts(w1, w2, w3):
    MhT = _box_mats()
    RT = _resize_mat()
    W1b = np.zeros((G * 6, 128), np.float32)   # [g*6+ci, g*32+co]
    W2b = np.zeros((128, 128), np.float32)     # [g*32+ci, g*32+co]
    W3b = np.zeros((128, G * 3), np.float32)   # [g*32+ci, g*3+co]
    for g in range(G):
        W1b[g * 6:(g + 1) * 6, g * 32:(g + 1) * 32] = w1.T
        W2b[g * 32:(g + 1) * 32, g * 32:(g + 1) * 32] = w2.T
        W3b[g * 32:(g + 1) * 32, g * 3:(g + 1) * 3] = w3.T
    S32 = np.zeros((128, 32), np.float32)      # sum over groups
    Sb = np.zeros((32, 128), np.float32)       # broadcast to groups
    for g in range(G):
        for co in range(32):
            S32[g * 32 + co, co] = 1.0 / 32.0
            Sb[co, g * 32 + co] = 1.0
    return dict(mht=MhT, rt=RT, w1b=W1b, w2b=W2b, w3b=W3b, s32=S32, sbc=Sb)


# ------------------------------------------------------------------ bass build
def _emit(nc, collectives=True, phases="AB"):
    lvl = int(phases[1]) if len(phases) > 1 and phases[1].isdigit() else 9
    xlr_d = nc.dram_tensor("xlr", [BS, C, n, n], F32, kind="ExternalInput")
    ylr_d = nc.dram_tensor("ylr", [BS, C, n, n], F32, kind="ExternalInput")
    hr_d = nc.dram_tensor("hr", [BS, C, N, N], F32, kind="ExternalInput")
    mht_d = nc.dram_tensor("mht", [n, n], F32, kind="ExternalInput")
    rt_d = nc.dram_tensor("rt", [n, N], F32, kind="ExternalInput")
    w1b_d = nc.dram_tensor("w1b", [G * 6, 128], F32, kind="ExternalInput")
    w2b_d = nc.dram_tensor("w2b", [128, 128], F32, kind="ExternalInput")
    w3b_d = nc.dram_tensor("w3b", [128, G * 3], F32, kind="ExternalInput")
    s32_d = nc.dram_tensor("s32", [128, 32], F32, kind="ExternalInput")
    sbc_d = nc.dram_tensor("sbc", [32, 128], F32, kind="ExternalInput")
    gb_d = nc.dram_tensor("gb", [32, 4], F32, kind="ExternalInput")  # g1 b1 g2 b2
    out_d = nc.dram_tensor("out", [BS, C, N, N], F32, kind="ExternalOutput")

    with tile.TileContext(nc) as tc, ExitStack() as ctx:
        consts = ctx.enter_context(tc.tile_pool(name="consts", bufs=1))
        persist = ctx.enter_context(tc.tile_pool(name="persist", bufs=1))
        statp = ctx.enter_context(tc.tile_pool(name="stats", bufs=1))
        dram = ctx.enter_context(tc.tile_pool(name="dram", bufs=1, space="DRAM"))

        # ---- constants into SBUF
        mht_s = consts.tile([n, n], F32, name="mht", tag="mht")
        rt_s = consts.tile([n, N], F32, name="rt", tag="rt")
        rt_r = consts.tile([n, N], F32R, name="rtr", tag="rtr")
        w1_r = consts.tile([G * 6, 128], F32, name="w1r", tag="w1r")
        w2_r = consts.tile([128, 128], F32, name="w2r", tag="w2r")
        w3_r = consts.tile([128, G * 3], F32, name="w3r", tag="w3r")
        s32_s = consts.tile([128, 32], F32, name="s32", tag="s32")
        sbc_s = consts.tile([32, 128], F32, name="sbc", tag="sbc")
        gb_s = consts.tile([32, 4], F32, name="gb", tag="gb")
        eps_s = consts.tile([32, 1], F32, name="eps", tag="eps")
        nc.gpsimd.dma_start(out=mht_s[:], in_=mht_d[:])
        nc.gpsimd.dma_start(out=w1_r[:], in_=w1b_d[:])
        nc.gpsimd.dma_start(out=w2_r[:], in_=w2b_d[:])
        nc.gpsimd.dma_start(out=w3_r[:], in_=w3b_d[:])
        nc.gpsimd.dma_start(out=s32_s[:], in_=s32_d[:])
        nc.gpsimd.dma_start(out=sbc_s[:], in_=sbc_d[:])
        nc.gpsimd.dma_start(out=gb_s[:], in_=gb_d[:])
        nc.gpsimd.dma_start(out=rt_s[:], in_=rt_d[:])
        nc.gpsimd.dma_start(out=rt_r[:], in_=rt_d[:])
        nc.vector.memset(eps_s[:], EPS)

        # persistent planes for the upsample phase
        a_sb = [persist.tile([n, C, n], F32, name=f"a{b}", tag=f"a{b}") for b in range(BS)]
        bp_sb = [persist.tile([n, C, n], F32, name=f"bp{b}", tag=f"bp{b}") for b in range(BS)]

        # BN stat tiles
        stats6 = [statp.tile([128, NT, 6], F32, name=f"st6{l}", tag=f"st6{l}")
                  for l in range(2)]
        bc_s = [statp.tile([128, 2], F32, name=f"bc{l}", tag=f"bc{l}") for l in range(2)]

        feat_dram = dram.tile([BS, 6, n, n], F32, name="featd", tag="featd")
        ab_dram = dram.tile([BS, C, n, n], F32, name="abd", tag="abd")
        ag_in = [dram.tile([32, 2], F32, name=f"agi{l}", tag=f"agi{l}") for l in range(2)]
        ag_out = [dram.tile([32 * N_CORES, 2], F32, name=f"ago{l}", tag=f"ago{l}") for l in range(2)]

        copy_rr = [0]  # round-robin ACT/DVE for psum->sbuf copies

        def pcopy(out_ap, in_ap):
            if copy_rr[0] % 2 == 0:
                nc.scalar.activation(out_ap, in_ap, ACTF.Copy)
            else:
                nc.vector.tensor_copy(out_ap, in_ap)
            copy_rr[0] += 1

        # ================= Phase A: lowres branch =================
        if phases == "B":
            for b in range(BS):
                nc.vector.memset(a_sb[b][:], 0.5)
                nc.vector.memset(bp_sb[b][:], 0.25)
        if phases != "B":
            with ExitStack() as actx:
                lrp = actx.enter_context(tc.tile_pool(name="lrp", bufs=1))
                prod = actx.enter_context(tc.tile_pool(name="prod", bufs=1))
                mtmp = actx.enter_context(tc.tile_pool(name="mtmp", bufs=2))
                rbp = actx.enter_context(tc.tile_pool(name="rbp", bufs=4))
                colp = actx.enter_context(tc.tile_pool(name="colp", bufs=3))
                featp = actx.enter_context(tc.tile_pool(name="featp", bufs=2))
                mlp = actx.enter_context(tc.tile_pool(name="mlp", bufs=1))
                tinyp = actx.enter_context(tc.tile_pool(name="tiny", bufs=2))


                x_sb, y_sb, mx_sb, my_sb = ({} for _ in range(4))
                for b in range(BS):
                    x_sb[b] = lrp.tile([n, C, n], F32, name=f"x{b}", tag=f"x{b}")
                    y_sb[b] = lrp.tile([n, C, n], F32, name=f"y{b}", tag=f"y{b}")
                    mx_sb[b] = lrp.tile([n, C, n], F32, name=f"mx{b}", tag=f"mx{b}")
                    my_sb[b] = lrp.tile([n, C, n], F32, name=f"my{b}", tag=f"my{b}")
                    nc.sync.dma_start(
                        out=x_sb[b][:], in_=xlr_d[b].rearrange("c h w -> h c w"))
                    nc.scalar.dma_start(
                        out=y_sb[b][:], in_=ylr_d[b].rearrange("c h w -> h c w"))

                def boxmean(src_t, dst_ap, eng):
                    """dst = rowbox (PE, Mh pre-scaled by 1/(3*nrow)) then
                    column 3-tap sum with edge columns rescaled by 3/2."""
                    p_bx = ps_box.tile([n, C * n], F32, name="pbx", tag="pbx")
                    nc.tensor.matmul(
                        p_bx[:], mht_s[:], src_t.rearrange("h c w -> h (c w)"),
                        start=True, stop=True)
                    rb = rbp.tile([n, C, n], F32, name="rb", tag="rb")
                    pcopy(rb[:], p_bx[:].rearrange("h (c w) -> h c w", c=C))
                    s1 = colp.tile([n, C, n - 2], F32, name="s1", tag="s1")
                    eng.tensor_add(s1[:], rb[:, :, 0:n - 2], rb[:, :, 1:n - 1])
                    eng.tensor_add(dst_ap[:, :, 1:n - 1], s1[:], rb[:, :, 2:n])
                    e0 = colp.tile([n, C, 1], F32, name="e0", tag="e0")
                    nc.vector.tensor_add(e0[:], rb[:, :, 0:1], rb[:, :, 1:2])
                    nc.vector.tensor_scalar_mul(dst_ap[:, :, 0:1], e0[:], 1.5)
                    e1 = colp.tile([n, C, 1], F32, name="e1", tag="e1")
                    nc.vector.tensor_add(e1[:], rb[:, :, n - 2:n - 1], rb[:, :, n - 1:n])
                    nc.vector.tensor_scalar_mul(dst_ap[:, :, n - 1:n], e1[:], 1.5)

                ps_box_ctx = tc.tile_pool(name="ps_box", bufs=6, space="PSUM")
                ps_box = ps_box_ctx.__enter__()
                for b in range(BS):
                    feat_s = featp.tile([n, 6, n], F32, name="feat", tag="feat")
                    xy_s = prod.tile([n, C, n], F32, name="xy", tag="xy")
                    xx_s = prod.tile([n, C, n], F32, name="xx", tag="xx")
                    mxy_s = mtmp.tile([n, C, n], F32, name="mxy", tag="mxy")
                    mxx_s = mtmp.tile([n, C, n], F32, name="mxx", tag="mxx")
                    nc.vector.tensor_mul(xy_s[:], x_sb[b][:], y_sb[b][:])
                    nc.gpsimd.tensor_mul(xx_s[:], x_sb[b][:], x_sb[b][:])
                    boxmean(x_sb[b][:], mx_sb[b], nc.vector)
                    boxmean(y_sb[b][:], my_sb[b], nc.gpsimd)
                    boxmean(xy_s[:], mxy_s, nc.vector)
                    boxmean(xx_s[:], mxx_s, nc.gpsimd)
                    # cov = mxy - mx*my ; var = mxx - mx*mx  (feat = [cov, var])
                    tprod = prod.tile([n, C, n], F32, name="tp", tag="tp")
                    tprod2 = prod.tile([n, C, n], F32, name="tp2", tag="tp2")
                    nc.vector.tensor_mul(tprod[:], mx_sb[b][:], my_sb[b][:])
                    nc.vector.tensor_sub(feat_s[:, 0:3, :], mxy_s[:], tprod[:])
                    nc.gpsimd.tensor_mul(tprod2[:], mx_sb[b][:], mx_sb[b][:])
                    nc.gpsimd.tensor_sub(feat_s[:, 3:6, :], mxx_s[:], tprod2[:])
                    nc.scalar.dma_start(
                        out=feat_dram[b].rearrange("c h w -> h c w"), in_=feat_s[:])

                ps_box_ctx.__exit__(None, None, None)
                ps_z = actx.enter_context(
                    tc.tile_pool(name="ps_z", bufs=6, space="PSUM"))
                ps_tiny = actx.enter_context(
                    tc.tile_pool(name="ps_tiny", bufs=1, space="PSUM"))
                if lvl < 2:
                    return nc.compile() or nc
                # ---- feat -> channel-major packed (fp32r) [24, BS, 32, n]
                feat_cm_b = []
                for b in range(BS):
                    fcb = mlp.tile([G * 6, 32, n], F32, name=f"fc{b}", tag="featcm")
                    for g in range(G):
                        nc.scalar.dma_start(
                            out=fcb[g * 6:(g + 1) * 6],
                            in_=feat_dram[b, :, g * 32:(g + 1) * 32, :])
                    feat_cm_b.append(fcb)

                def feat_rhs(t):
                    half = feat_cm_b[t // (NT // 2)]
                    return half.rearrange("q r w -> q (r w)")[
                        :, bass.ts(t % (NT // 2), PT)]

                z1_r = mlp.tile([128, PF], F32, name="z1", tag="z1")
                z2_r = mlp.tile([128, PF], F32, name="z2", tag="z2")

                def conv_layer(l, w_r, rhs_fn, z_out):
                    """matmul w_r.T@rhs per tile; ACT/DVE copy to SBUF; bn_stats."""
                    for t in range(NT):
                        sl = bass.ts(t, PT)
                        p_z = ps_z.tile([128, PT], F32, name="pz", tag="pz")
                        nc.tensor.matmul(
                            p_z[:], w_r[:], rhs_fn(t), start=True, stop=True)
                        pcopy(z_out[:, sl], p_z[:])
                        nc.vector.bn_stats(out=stats6[l][:, t, :], in_=p_z[:])

                def bn_stats_to_scalebias(l, g_col, b_col):
                    """per-partition mean/E[z^2] -> AllGather -> scale/bias."""
                    mv = tinyp.tile([128, 2], F32, name="mv", tag="mv")
                    nc.vector.bn_aggr(out=mv[:], in_=stats6[l][:])
                    mm2l = tinyp.tile([128, 1], F32, name="mm2l", tag="mm2l")
                    nc.vector.tensor_mul(mm2l[:], mv[:, 0:1], mv[:, 0:1])
                    loc2 = tinyp.tile([128, 2], F32, name="loc2", tag="loc2")
                    nc.vector.tensor_copy(loc2[:, 0:1], mv[:, 0:1])
                    nc.vector.tensor_add(loc2[:, 1:2], mv[:, 1:2], mm2l[:])
                    p_st = ps_tiny.tile([32, 2], F32, name="pst", tag="pst")
                    nc.tensor.matmul(p_st[:], s32_s[:], loc2[:], start=True, stop=True)
                    st_s = tinyp.tile([32, 2], F32, name="sts", tag="sts")
                    nc.vector.tensor_copy(st_s[:], p_st[:])
                    nc.scalar.dma_start(out=ag_in[l][:], in_=st_s[:])
                    if collectives:
                        nc.gpsimd.collective_compute(
                            "AllGather", ALU.bypass,
                            replica_groups=[list(range(N_CORES))],
                            ins=[ag_in[l][:].opt()], outs=[ag_out[l][:].opt()])
                    else:  # timing-only stand-in for the collective
                        nc.gpsimd.dma_start(
                            out=ag_out[l][0:32, :], in_=ag_in[l][:])
                    g_s = tinyp.tile([32, 2, N_CORES], F32, name="gs", tag="gs")
                    nc.scalar.dma_start(
                        out=g_s[:],
                        in_=ag_out[l][:].rearrange("(r p) s -> p s r", p=32))
                    red = tinyp.tile([32, 2], F32, name="red", tag="red")
                    nc.vector.tensor_reduce(
                        out=red[:], in_=g_s[:], axis=AX.X, op=ALU.add)
                    m_s = red[:, 0:1]
                    v_s = tinyp.tile([32, 1], F32, name="vs", tag="vs")
                    mm_s = tinyp.tile([32, 1], F32, name="mms", tag="mms")
                    sb2 = tinyp.tile([32, 2], F32, name="sb2", tag="sb2")
                    nc.vector.tensor_mul(mm_s[:], m_s, m_s)
                    nc.vector.tensor_sub(v_s[:], red[:, 1:2], mm_s[:])
                    sd_s = tinyp.tile([32, 1], F32, name="sds", tag="sds")
                    nc.scalar.activation(sd_s[:], v_s[:], ACTF.Sqrt, bias=eps_s[:])
                    nc.vector.reciprocal(sd_s[:], sd_s[:])
                    # scale = g * rinv ; bias = b - m*scale
                    nc.vector.tensor_mul(sb2[:, 0:1], gb_s[:, g_col:g_col + 1], sd_s[:])
                    nc.vector.tensor_mul(mm_s[:], m_s, sb2[:, 0:1])
                    nc.vector.tensor_sub(sb2[:, 1:2], gb_s[:, b_col:b_col + 1], mm_s[:])
                    p_bc = ps_tiny.tile([128, 2], F32, name="pbc", tag="pbc")
                    nc.tensor.matmul(p_bc[:], sbc_s[:], sb2[:], start=True, stop=True)
                    nc.vector.tensor_copy(bc_s[l][:], p_bc[:])

                def relu_pass(l, z_io):
                    for t in range(NT):
                        sl = bass.ts(t, PT)
                        nc.scalar.activation(
                            z_io[:, sl], z_io[:, sl], ACTF.Relu,
                            bias=bc_s[l][:, 1:2], scale=bc_s[l][:, 0:1])

                if lvl < 3:
                    return nc.compile() or nc
                conv_layer(0, w1_r, feat_rhs, z1_r)
                bn_stats_to_scalebias(0, 0, 1)
                relu_pass(0, z1_r)
                if lvl < 4:
                    return nc.compile() or nc
                conv_layer(1, w2_r, lambda t: z1_r[:, bass.ts(t, PT)], z2_r)
                bn_stats_to_scalebias(1, 2, 3)
                relu_pass(1, z2_r)
                if lvl < 5:
                    return nc.compile() or nc

                # conv3 -> A packed -> DRAM planes -> b planes, per sample
                for b in range(BS):
                    apk_b = mlp.tile([G * 3, 32, n], F32, name=f"ap{b}", tag="apk")
                    apk_f = apk_b.rearrange("q r w -> q (r w)")
                    for t in range(b * NT // 2, (b + 1) * NT // 2):
                        sl = bass.ts(t, PT)
                        p_a = ps_z.tile([G * 3, PT], F32, name="pa", tag="pz")
                        nc.tensor.matmul(
                            p_a[:], w3_r[:], z2_r[:, sl], start=True, stop=True)
                        pcopy(apk_f[:, bass.ts(t % (NT // 2), PT)], p_a[:])
                    for g in range(G):
                        nc.scalar.dma_start(
                            out=ab_dram[b, :, g * 32:(g + 1) * 32, :],
                            in_=apk_b[g * 3:(g + 1) * 3])
                    nc.scalar.dma_start(
                        out=a_sb[b][:], in_=ab_dram[b].rearrange("c h w -> h c w"))
                    # b = my - A * mx
                    tpb = prod.tile([n, C, n], F32, name="tp", tag="tp")
                    nc.vector.tensor_mul(tpb[:], a_sb[b][:], mx_sb[b][:])
                    nc.vector.tensor_sub(bp_sb[b][:], my_sb[b][:], tpb[:])

        # ================= Phase B: upsample + fuse =================
        if phases != "A":
            with ExitStack() as uctx:
                t1rp = uctx.enter_context(tc.tile_pool(name="t1rp", bufs=8))
                hrp = uctx.enter_context(tc.tile_pool(name="hrp", bufs=10))
                outp = uctx.enter_context(tc.tile_pool(name="outp", bufs=8))
                bubp = uctx.enter_context(tc.tile_pool(name="bubp", bufs=3))
                hrp2 = hrp
                ps_up = uctx.enter_context(
                    tc.tile_pool(name="ps_up", bufs=4, space="PSUM"))

                for b in range(BS):
                    for c in range(C):
                        t1s = {}
                        for key, srcp in (("a", a_sb[b]), ("b", bp_sb[b])):
                            p_t1 = ps_up.tile([n, N], F32, name="psu", tag="psu")
                            for h in range(2):
                                nc.tensor.matmul(
                                    p_t1[:, bass.ts(h, 512)], srcp[:, c, :],
                                    rt_s[:, bass.ts(h, 512)], start=True, stop=True)
                            t1_r = t1rp.tile([n, N], F32R, name="t1r", tag="t1r")
                            nc.scalar.activation(t1_r[:], p_t1[:], ACTF.Copy)
                            t1s[key] = t1_r
                        for blk in range(BLK):
                            p_ua = ps_up.tile([n, N], F32, name="psu", tag="psu")
                            p_ub = ps_up.tile([n, N], F32, name="psu", tag="psu")
                            for h in range(2):
                                hs = bass.ts(h, 512)
                                nc.tensor.matmul(
                                    p_ua[:, hs], t1s["a"][:, bass.ts(blk, 128)],
                                    rt_r[:, hs], start=True, stop=True)
                                nc.tensor.matmul(
                                    p_ub[:, hs], t1s["b"][:, bass.ts(blk, 128)],
                                    rt_r[:, hs], start=True, stop=True)
                            fuse_i = (b * C + c) * BLK + blk
                            hp = hrp if fuse_i % 2 == 0 else hrp2
                            hr_s = hp.tile([n, N], F32, name="hr", tag="hr")
                            nc.sync.dma_start(
                                out=hr_s[:],
                                in_=hr_d[b, c, blk * 128:(blk + 1) * 128, :])
                            o_s = outp.tile([n, N], F32, name="o", tag="o")
                            if fuse_i % 3 == 2:  # scheme C: ACT copy + GPS add
                                bub = bubp.tile([n, N], F32, name="bub", tag="bub")
                                nc.scalar.activation(bub[:], p_ub[:], ACTF.Copy)
                                nc.vector.tensor_mul(o_s[:], p_ua[:], hr_s[:])
                                nc.gpsimd.tensor_add(o_s[:], o_s[:], bub[:])
                            else:
                                nc.vector.tensor_mul(o_s[:], p_ua[:], hr_s[:])
                                nc.vector.tensor_add(o_s[:], o_s[:], p_ub[:])
                            nc.scalar.dma_start(
                                out=out_d[b, c, blk * 128:(blk + 1) * 128, :],
                                in_=o_s[:])
    nc.compile()
    return nc


_NC = None


def _get_nc():
    global _NC
    if _NC is None:
        ncb = bacc.Bacc(
            "TRN2", target_bir_lowering=False, debug=False,
            num_devices=N_CORES)
        _NC = _emit(ncb)
    return _NC


def kernel(image_lr, guide_lr, image_hr, w_box, w1, g1, b1, w2, g2, b2, w3):
    image_lr = np.ascontiguousarray(np.asarray(image_lr, np.float32))
    guide_lr = np.ascontiguousarray(np.asarray(guide_lr, np.float32))
    image_hr = np.ascontiguousarray(np.asarray(image_hr, np.float32))
    consts = _host_consts(np.asarray(w1, np.float32),
                          np.asarray(w2, np.float32),
                          np.asarray(w3, np.float32))
    gb = np.stack([np.asarray(v, np.float32) for v in (g1, b1, g2, b2)],
                  axis=1)  # [32, 4]
    nc = _get_nc()
    in_maps = []
    for i in range(N_CORES):
        sl = slice(i * BS, (i + 1) * BS)
        m = dict(xlr=image_lr[sl], ylr=guide_lr[sl], hr=image_hr[sl], gb=gb)
        m.update({k: np.ascontiguousarray(v) for k, v in consts.items()})
        in_maps.append(m)
    res = run_bass_kernel_spmd(nc, in_maps, core_ids=list(range(N_CORES)))
    global LAST_RESULT
    LAST_RESULT = res
    out = np.concatenate([res.results[i]["out"] for i in range(N_CORES)], 0)
    return out.astype(np.float32)


LAST_RESULT = None



# revision 26
# speedup vs baseline: 1.1477x; 1.1477x over previous
"""ConvGuidedFilter Trainium2 kernel (8 NeuronCores, batch-parallel).

v2 strategy (single-core timeline optimized around the 360 GB/s DMA roofline):
- Shard batch 16 -> 2 samples per core; BN batch stats AllGather'd across
  cores (local stats fail the accuracy gate).
- hr prefetch ring fills the DMA engines from t=0 while the lowres phase runs.
- Box filter = row matmul (normalization folded) + column 3-tap adds.
- 1x1-conv MLP over channel-major packed pixels; all big matmuls use fp32r
  moving tensors (bitcast, 4x faster than fp32 rows); conv2 is recomputed
  after BN2 stats instead of storing z2.
- Upsample stage2 accumulates b_up (PE, start=False) onto A_up*hr computed by
  DVE straight into PSUM; ACT copies PSUM->SBUF; SP issues paired DMAs.
"""
import os
import sys

for _p in ("/opt/trn_rl_repo", "/root/.axon_site/_ro/trn_rl_repo"):
    if os.path.isdir(_p) and _p not in sys.path:
        sys.path.insert(0, _p)

from contextlib import ExitStack

import numpy as np
import concourse.bass as bass
import concourse.tile as tile
from concourse import bacc, mybir
from concourse.bass_utils import run_bass_kernel_spmd

F32 = mybir.dt.float32
F32R = mybir.dt.float32r
AX = mybir.AxisListType
ALU = mybir.AluOpType
ACTF = mybir.ActivationFunctionType

B, C, n, N = 16, 3, 128, 1024  # batch, channels, lowres, hires
N_CORES, BS = 8, 2             # cores, samples per core
G = 4                          # pixel groups (32 lowres rows each)
PF = BS * 32 * n               # packed free size per partition = 8192
PT = 1024                      # pixel tile (free)
NT = PF // PT                  # 8 tiles
EPS = 1e-5
BLK = 8                        # hires row blocks per plane (1024/128)
NPLANE = BS * C                # 6 planes per core
NSUP = NPLANE * BLK // 2       # 24 paired hr/out transfers
R_SUP = 12                     # hr ring depth in supertiles (2 blocks each)


# ---------------------------------------------------------------- host consts
def _box_mats():
    Bm = np.zeros((n, n), np.float32)
    for i in range(n):
        Bm[i, max(0, i - 1):min(n, i + 2)] = 1.0
    cnt = Bm.sum(1)
    # row-box matrix with the interior normalization 1/(3*nrow) folded in;
    # edge *columns* get a 3/2 fixup after the column 3-tap sum.
    Mh = (Bm / (3.0 * cnt[:, None])).astype(np.float32)
    return np.ascontiguousarray(Mh.T)  # [h_in, h_out]


def _resize_mat():
    c = np.arange(N, dtype=np.float32) * ((n - 1) / (N - 1))
    i0 = np.clip(np.floor(c).astype(np.int64), 0, n - 2)
    t = (c - i0).astype(np.float32)
    R = np.zeros((N, n), np.float32)
    R[np.arange(N), i0] = 1.0 - t
    R[np.arange(N), i0 + 1] += t
    return np.ascontiguousarray(R.T)  # [n_in=128, n_out=1024]


def _host_consts(w1, w2, w3):
    W1b = np.zeros((64, 128), np.float32)      # [b*32 + ci*4+g, g*32+co]
    W2b = np.zeros((128, 128), np.float32)     # [g*32+ci, g*32+co]
    W3b = np.zeros((128, G * C), np.float32)   # [g*32+ci, c*4+g]
    for g in range(G):
        for ci in range(6):
            W1b[ci * G + g, g * 32:(g + 1) * 32] = w1[:, ci]
            W1b[32 + ci * G + g, g * 32:(g + 1) * 32] = w1[:, ci]
        W2b[g * 32:(g + 1) * 32, g * 32:(g + 1) * 32] = w2.T
        for c in range(C):
            W3b[g * 32:(g + 1) * 32, c * G + g] = w3[c, :]
    S32 = np.zeros((128, 32), np.float32)      # sum over groups&cores /32
    Sb = np.zeros((32, 128), np.float32)       # broadcast to groups
    for g in range(G):
        for co in range(32):
            S32[g * 32 + co, co] = 1.0 / 32.0
            Sb[co, g * 32 + co] = 1.0
    return dict(mht=_box_mats(), rt=_r32(_resize_mat()),
                w1b=_r32(W1b), w2b=_r32(W2b), w3b=_r32(W3b), s32=S32, sbc=Sb)


def _r32(x):
    # round fp32 -> fp32r-representable (bf16 hi + bf16 lo)
    import ml_dtypes
    hi = x.astype(ml_dtypes.bfloat16).astype(np.float32)
    lo = (x - hi).astype(ml_dtypes.bfloat16).astype(np.float32)
    return np.ascontiguousarray(hi + lo)


# ------------------------------------------------------------------ bass build
def _emit(nc, collectives=True, phases="AB"):
    xlr_d = nc.dram_tensor("xlr", [BS, C, n, n], F32, kind="ExternalInput")
    ylr_d = nc.dram_tensor("ylr", [BS, C, n, n], F32, kind="ExternalInput")
    hr_d = nc.dram_tensor("hr", [BS, C, N, N], F32, kind="ExternalInput")
    mht_d = nc.dram_tensor("mht", [n, n], F32, kind="ExternalInput")
    rt_d = nc.dram_tensor("rt", [n, N], F32R, kind="ExternalInput")
    w1b_d = nc.dram_tensor("w1b", [64, 128], F32R, kind="ExternalInput")
    w2b_d = nc.dram_tensor("w2b", [128, 128], F32R, kind="ExternalInput")
    w3b_d = nc.dram_tensor("w3b", [128, G * 3], F32R, kind="ExternalInput")
    s32_d = nc.dram_tensor("s32", [128, 32], F32, kind="ExternalInput")
    sbc_d = nc.dram_tensor("sbc", [32, 128], F32, kind="ExternalInput")
    gb_d = nc.dram_tensor("gb", [32, 4], F32, kind="ExternalInput")  # g1 b1 g2 b2
    out_d = nc.dram_tensor("out", [BS, C, N, N], F32, kind="ExternalOutput")

    with tile.TileContext(nc) as tc, ExitStack() as ctx:
        consts = ctx.enter_context(tc.tile_pool(name="consts", bufs=1))
        persist = ctx.enter_context(tc.tile_pool(name="persist", bufs=1))
        ringp = ctx.enter_context(tc.tile_pool(name="ring", bufs=R_SUP))
        statp = ctx.enter_context(tc.tile_pool(name="stats", bufs=1))
        dram = ctx.enter_context(tc.tile_pool(name="dram", bufs=1, space="DRAM"))

        # ---- constants into SBUF (ACT queue; all tiny except rt)
        mht_s = consts.tile([n, n], F32, name="mht", tag="mht")
        rt_s = consts.tile([n, N], F32R, name="rt", tag="rt")
        w1_s = consts.tile([64, 128], F32R, name="w1s", tag="w1s")
        w2_s = consts.tile([128, 128], F32R, name="w2s", tag="w2s")
        w3_s = consts.tile([128, G * 3], F32R, name="w3s", tag="w3s")
        s32_s = consts.tile([128, 32], F32, name="s32", tag="s32")
        sbc_s = consts.tile([32, 128], F32, name="sbc", tag="sbc")
        gb_s = consts.tile([32, 4], F32, name="gb", tag="gb")
        eps_s = consts.tile([32, 1], F32, name="eps", tag="eps")
        nc.sync.dma_start(out=mht_s[:], in_=mht_d[:])
        nc.scalar.dma_start(out=rt_s[:], in_=rt_d[:])
        nc.vector.memset(eps_s[:], EPS)
        warm_s = consts.tile([32, 1], F32, name="warm", tag="warm")
        nc.scalar.activation(warm_s[:], eps_s[:, 0:1], ACTF.Sqrt)

        # ---- lowres inputs, both samples per DMA (SP queue, first)
        x_sb = persist.tile([n, BS, C, n], F32, name="x", tag="x")
        y_sb = persist.tile([n, BS, C, n], F32, name="y", tag="y")
        nc.sync.dma_start(out=x_sb[:], in_=xlr_d.rearrange("b c h w -> h b c w"))
        nc.sync.dma_start(out=y_sb[:], in_=ylr_d.rearrange("b c h w -> h b c w"))
        for dst, srcd in ((w1_s, w1b_d), (w2_s, w2b_d), (w3_s, w3b_d),
                          (s32_s, s32_d), (sbc_s, sbc_d), (gb_s, gb_d)):
            nc.sync.dma_start(out=dst[:], in_=srcd[:])

        # ---- hr prefetch ring: 24 paired loads. First 3 + the ring-WAR
        # self-paced tail go on SP at t=0; loads 3..R_SUP-1 are issued from
        # the ACT queue at milestones inside phase A so the shared DMA-engine
        # queue stays short for latency-critical small transfers.
        hr_t = [ringp.tile([n, 2, N], F32, name=f"hr{j}", tag="hr")
                for j in range(NSUP)]

        def hr_load(j, eng):
            p, k = j // (BLK // 2), j % (BLK // 2)
            eng.dma_start(
                out=hr_t[j][:],
                in_=hr_d[p // C, p % C, k * 256:(k + 1) * 256].rearrange(
                    "(two h) w -> h two w", two=2))

        # pace the prefetch: schedule load j no earlier than ~3us*j so the
        # shared DMA-engine queue stays short for latency-critical small
        # transfers woven through phase A.
        for j in range(R_SUP):
            t_j = 0.00295 * j if j < 6 else 0.0177 + 0.0042 * (j - 6)
            with tc.tile_wait_until(t_j, enable=j >= 2):
                hr_load(j, nc.sync)

        # persistent lowres planes
        a_sb = [persist.tile([n, C, n], F32R, name=f"a{b}", tag=f"a{b}")
                for b in range(BS)]
        bp_sb = [persist.tile([n, C, n], F32R, name=f"bp{b}", tag=f"bp{b}")
                 for b in range(BS)]
        mx_sb = [persist.tile([n, C, n], F32, name=f"mx{b}", tag=f"mx{b}")
                 for b in range(BS)]
        my_sb = [persist.tile([n, C, n], F32, name=f"my{b}", tag=f"my{b}")
                 for b in range(BS)]

        stats6 = [statp.tile([128, 2 * NT, 6], F32, name=f"st6{l}",
                             tag=f"st6{l}") for l in range(2)]
        bc_s = [statp.tile([128, 2], F32, name=f"bc{l}", tag=f"bc{l}")
                for l in range(2)]

        feat_dram = dram.tile([BS, 6, n, n], F32R, name="featd", tag="featd")
        ab_dram = dram.tile([BS, C, n, n], F32R, name="abd", tag="abd")
        ag_in = [dram.tile([32, 2], F32, name=f"agi{l}", tag=f"agi{l}")
                 for l in range(2)]
        ag_out = [dram.tile([32 * N_CORES, 2], F32, name=f"ago{l}",
                            tag=f"ago{l}") for l in range(2)]

        # ================= Phase A: lowres branch =================
        if phases == "B":
            for b in range(BS):
                nc.vector.memset(a_sb[b][:], 0.5)
                nc.vector.memset(bp_sb[b][:], 0.25)
        if phases != "B":
            with ExitStack() as actx:
                prodp = actx.enter_context(tc.tile_pool(name="prod", bufs=1))
                rbp = actx.enter_context(tc.tile_pool(name="rbp", bufs=2))
                colp = actx.enter_context(tc.tile_pool(name="colp", bufs=2))
                featp = actx.enter_context(tc.tile_pool(name="featp", bufs=1))
                mlp = actx.enter_context(tc.tile_pool(name="mlp", bufs=1))
                znp = actx.enter_context(tc.tile_pool(name="znp", bufs=2))
                tinyp = actx.enter_context(tc.tile_pool(name="tiny", bufs=2))

                ps_box_ctx = tc.tile_pool(name="ps_box", bufs=4, space="PSUM")
                ps_box = ps_box_ctx.__enter__()

                def boxmean(src_ap, dst_ap, eng):
                    """dst = rowbox (PE, Mh pre-scaled) then column 3-tap sum
                    with edge columns rescaled by 3/2."""
                    p_bx = ps_box.tile([n, C * n], F32, name="pbx", tag="pbx")
                    nc.tensor.matmul(
                        p_bx[:], mht_s[:],
                        src_ap.rearrange("h c w -> h (c w)"),
                        start=True, stop=True)
                    rb = rbp.tile([n, C, n], F32, name="rb", tag="rb")
                    nc.scalar.activation(
                        rb[:], p_bx[:].rearrange("h (c w) -> h c w", c=C),
                        ACTF.Copy)
                    s1 = colp.tile([n, C, n - 2], F32, name="s1", tag="s1")
                    eng.tensor_add(s1[:], rb[:, :, 0:n - 2], rb[:, :, 1:n - 1])
                    eng.tensor_add(dst_ap[:, :, 1:n - 1], s1[:], rb[:, :, 2:n])
                    e0 = colp.tile([n, C, 1], F32, name="e0", tag="e0")
                    eng.tensor_add(e0[:], rb[:, :, 0:1], rb[:, :, 1:2])
                    eng.tensor_scalar_mul(dst_ap[:, :, 0:1], e0[:], 1.5)
                    e1 = colp.tile([n, C, 1], F32, name="e1", tag="e1")
                    eng.tensor_add(e1[:], rb[:, :, n - 2:n - 1], rb[:, :, n - 1:n])
                    eng.tensor_scalar_mul(dst_ap[:, :, n - 1:n], e1[:], 1.5)

                # fcb: channel-major packed feat, both samples,
                # partition q = b*24 + ci*4 + g (single-DMA transpose load)
                fcb_s = mlp.tile([64, 32, n], F32R, name="fcb", tag="fcb")
                z1_s = mlp.tile([128, PF], F32R, name="z1", tag="z1")
                apk_s = mlp.tile([64, 32, n], F32R, name="apk", tag="apk")

                for b in range(BS):
                    feat_s = featp.tile([n, 6, n], F32R, name="feat", tag="feat")
                    xy_s = prodp.tile([n, C, n], F32, name="xy", tag="p1")
                    xx_s = prodp.tile([n, C, n], F32, name="xx", tag="p2")
                    nc.vector.tensor_mul(xy_s[:], x_sb[:, b], y_sb[:, b])
                    nc.gpsimd.tensor_mul(xx_s[:], x_sb[:, b], x_sb[:, b])
                    boxmean(x_sb[:, b], mx_sb[b], nc.vector)
                    boxmean(y_sb[:, b], my_sb[b], nc.gpsimd)
                    mxy_s = prodp.tile([n, C, n], F32, name="mxy", tag="p1")
                    mxx_s = prodp.tile([n, C, n], F32, name="mxx", tag="p2")
                    boxmean(xy_s[:], mxy_s, nc.vector)
                    boxmean(xx_s[:], mxx_s, nc.gpsimd)
                    # cov = mxy - mx*my ; var = mxx - mx*mx  (feat = [cov, var])
                    tprod = prodp.tile([n, C, n], F32, name="tp", tag="p3")
                    nc.vector.tensor_mul(tprod[:], mx_sb[b][:], my_sb[b][:])
                    nc.vector.tensor_sub(feat_s[:, 0:3, :], mxy_s[:], tprod[:])
                    tprod2 = prodp.tile([n, C, n], F32, name="tp2", tag="p3")
                    nc.gpsimd.tensor_mul(tprod2[:], mx_sb[b][:], mx_sb[b][:])
                    nc.gpsimd.tensor_sub(feat_s[:, 3:6, :], mxx_s[:], tprod2[:])
                    nc.scalar.dma_start(
                        out=feat_dram[b].rearrange("c h w -> h c w"),
                        in_=feat_s[:])
                    nc.scalar.dma_start(
                        out=fcb_s[b * 32:b * 32 + G * 6],
                        in_=feat_dram[b].rearrange(
                            "c (g r) w -> (c g) r w", g=G))

                ps_box_ctx.__exit__(None, None, None)
                ps_z_ctx = tc.tile_pool(name="ps_z", bufs=3, space="PSUM")
                ps_z = ps_z_ctx.__enter__()
                ps_tiny_ctx = tc.tile_pool(name="ps_tiny", bufs=1, space="PSUM")
                ps_tiny = ps_tiny_ctx.__enter__()

                def feat_rhs(t):
                    b = t // (NT // 2)
                    half = fcb_s[b * 32:b * 32 + G * 6]
                    return half.rearrange("q r w -> q (r w)")[
                        :, bass.ts(t % (NT // 2), PT)]

                # conv1: matmul -> ACT copy to z1, DVE bn_stats on psum
                for t in range(NT):
                    p_z = ps_z.tile([128, PT], F32, name="pz", tag="pz")
                    for h in range(2):
                        hs = bass.ts(h, 512)
                        nc.tensor.matmul(
                            p_z[:, hs],
                            w1_s[t // (NT // 2) * 32:
                                 t // (NT // 2) * 32 + G * 6],
                            feat_rhs(t)[:, hs],
                            start=True, stop=True)
                    nc.scalar.activation(z1_s[:, bass.ts(t, PT)], p_z[:],
                                         ACTF.Copy)
                    for h in range(2):
                        nc.vector.bn_stats(out=stats6[0][:, 2 * t + h, :],
                                           in_=p_z[:, bass.ts(h, 512)])

                def bn_stats_to_scalebias(l, g_col, b_col):
                    """per-partition mean/E[z^2] -> AllGather -> scale/bias."""
                    mv = tinyp.tile([128, 2], F32, name="mv", tag="mv")
                    nc.vector.bn_aggr(out=mv[:], in_=stats6[l][:])
                    mm2l = tinyp.tile([128, 1], F32, name="mm2l", tag="mm2l")
                    nc.vector.tensor_mul(mm2l[:], mv[:, 0:1], mv[:, 0:1])
                    loc2 = tinyp.tile([128, 2], F32, name="loc2", tag="loc2")
                    nc.vector.tensor_copy(loc2[:, 0:1], mv[:, 0:1])
                    nc.vector.tensor_add(loc2[:, 1:2], mv[:, 1:2], mm2l[:])
                    p_st = ps_tiny.tile([32, 2], F32, name="pst", tag="pt")
                    nc.tensor.matmul(p_st[:], s32_s[:], loc2[:],
                                     start=True, stop=True)
                    st_s = tinyp.tile([32, 2], F32, name="sts", tag="sts")
                    nc.vector.tensor_copy(st_s[:], p_st[:])
                    nc.scalar.dma_start(out=ag_in[l][:], in_=st_s[:])
                    if collectives:
                        nc.gpsimd.collective_compute(
                            "AllGather", ALU.bypass,
                            replica_groups=[list(range(N_CORES))],
                            ins=[ag_in[l][:].opt()], outs=[ag_out[l][:].opt()])
                    else:  # timing-only stand-in for the collective
                        nc.gpsimd.dma_start(
                            out=ag_out[l][0:32, :], in_=ag_in[l][:])
                    g_s = tinyp.tile([32, 2, N_CORES], F32, name="gs", tag="gs")
                    nc.scalar.dma_start(
                        out=g_s[:],
                        in_=ag_out[l][:].rearrange("(r p) s -> p s r", p=32))
                    red = tinyp.tile([32, 2], F32, name="red", tag="red")
                    nc.vector.tensor_reduce(
                        out=red[:], in_=g_s[:], axis=AX.X, op=ALU.add)
                    m_s = red[:, 0:1]
                    v_s = tinyp.tile([32, 1], F32, name="vs", tag="vs")
                    mm_s = tinyp.tile([32, 1], F32, name="mms", tag="mms")
                    sb2 = tinyp.tile([32, 2], F32, name="sb2", tag="sb2")
                    nc.vector.tensor_mul(mm_s[:], m_s, m_s)
                    nc.vector.tensor_sub(v_s[:], red[:, 1:2], mm_s[:])
                    sd_s = tinyp.tile([32, 1], F32, name="sds", tag="sds")
                    nc.scalar.activation(sd_s[:], v_s[:], ACTF.Sqrt,
                                         bias=eps_s[:])
                    nc.vector.reciprocal(sd_s[:], sd_s[:])
                    # scale = g * rinv ; bias = b - m*scale
                    nc.vector.tensor_mul(sb2[:, 0:1],
                                         gb_s[:, g_col:g_col + 1], sd_s[:])
                    nc.vector.tensor_mul(mm_s[:], m_s, sb2[:, 0:1])
                    nc.vector.tensor_sub(sb2[:, 1:2],
                                         gb_s[:, b_col:b_col + 1], mm_s[:])
                    p_bc = ps_tiny.tile([128, 2], F32, name="pbc", tag="pt")
                    nc.tensor.matmul(p_bc[:], sbc_s[:], sb2[:],
                                     start=True, stop=True)
                    nc.vector.tensor_copy(bc_s[l][:], p_bc[:])

                bn_stats_to_scalebias(0, 0, 1)

                # relu1 in place (ACT), conv2 (PE), bn_stats2 (DVE) — no z2
                for t in range(NT):
                    sl = bass.ts(t, PT)
                    nc.scalar.activation(z1_s[:, sl], z1_s[:, sl], ACTF.Relu,
                                         bias=bc_s[0][:, 1:2],
                                         scale=bc_s[0][:, 0:1])
                    p_z = ps_z.tile([128, PT], F32, name="pz", tag="pz")
                    for h in range(2):
                        hs = bass.ts(h, 512)
                        nc.tensor.matmul(p_z[:, hs], w2_s[:],
                                         z1_s[:, sl][:, hs],
                                         start=True, stop=True)
                    for h in range(2):
                        nc.vector.bn_stats(out=stats6[1][:, 2 * t + h, :],
                                           in_=p_z[:, bass.ts(h, 512)])

                bn_stats_to_scalebias(1, 2, 3)

                # conv2 recompute -> fused scale/bias/relu copy -> conv3
                # -> apk (partition q = c*4+g); per-sample transpose via DRAM
                apk_f = apk_s.rearrange("q r w -> q (r w)")
                for t in range(NT):
                    sl = bass.ts(t, PT)
                    b = t // (NT // 2)
                    p_z = ps_z.tile([128, PT], F32, name="pz", tag="pz")
                    for h in range(2):
                        hs = bass.ts(h, 512)
                        nc.tensor.matmul(p_z[:, hs], w2_s[:],
                                         z1_s[:, sl][:, hs],
                                         start=True, stop=True)
                    zn2 = znp.tile([128, PT], F32R, name="zn2", tag="zn2")
                    nc.scalar.activation(zn2[:], p_z[:], ACTF.Relu,
                                         bias=bc_s[1][:, 1:2],
                                         scale=bc_s[1][:, 0:1])
                    p_a = ps_z.tile([G * C, PT], F32, name="pa", tag="pz")
                    for h in range(2):
                        hs = bass.ts(h, 512)
                        nc.tensor.matmul(p_a[:, hs], w3_s[:],
                                         zn2[:, hs], start=True, stop=True)
                    tq = t % (NT // 2)
                    nc.vector.tensor_copy(
                        apk_f[b * 32:b * 32 + G * C, bass.ts(tq, PT)],
                        p_a[:])
                    if tq == NT // 2 - 1:
                        # this sample's A is complete: transpose to [h, c, w]
                        nc.gpsimd.dma_start(
                            out=ab_dram[b].rearrange(
                                "c (g r) w -> (c g) r w", g=G),
                            in_=apk_s[b * 32:b * 32 + G * C])
                        nc.scalar.dma_start(
                            out=a_sb[b][:],
                            in_=ab_dram[b].rearrange("c h w -> h c w"))
                        tpb = prodp.tile([n, C, n], F32, name="tpb", tag="p3")
                        nc.vector.tensor_mul(tpb[:], a_sb[b][:].bitcast(F32), mx_sb[b][:])
                        nc.vector.tensor_sub(bp_sb[b][:], my_sb[b][:], tpb[:])

                # PE p-state warmer: dependency-free junk matmuls drain
                # whenever the PE is otherwise idle, keeping the clock ramp
                # hot through phase A's gaps (cold matmuls cost 3.7x).
                ps_w_ctx = tc.tile_pool(name="ps_w", bufs=1, space="PSUM")
                ps_w = ps_w_ctx.__enter__()
                xflat = x_sb.rearrange("h b c w -> h (b c w)")
                for _j in range(110):
                    p_w = ps_w.tile([64, 384], F32, name="pw", tag="pw")
                    nc.tensor.matmul(p_w[:], mht_s[:, 0:64], xflat[:, 0:384],
                                     start=True, stop=True)
                ps_w_ctx.__exit__(None, None, None)
                ps_tiny_ctx.__exit__(None, None, None)
                ps_z_ctx.__exit__(None, None, None)

        # ================= Phase B: upsample + fuse =================
        if phases != "A":
            with ExitStack() as uctx:
                t1rp = uctx.enter_context(tc.tile_pool(name="t1rp", bufs=4))
                outp = uctx.enter_context(tc.tile_pool(name="outp", bufs=3))
                ps_up = uctx.enter_context(
                    tc.tile_pool(name="ps_up", bufs=4, space="PSUM"))

                def stage1(p):
                    b, c = p // C, p % C
                    t1s = {}
                    for key, srcp in (("a", a_sb[b]), ("b", bp_sb[b])):
                        p_t1 = ps_up.tile([n, N], F32, name="psu", tag="psu")
                        for h in range(2):
                            hs = bass.ts(h, 512)
                            nc.tensor.matmul(p_t1[:, hs], srcp[:, c, :],
                                             rt_s[:, hs],
                                             start=True, stop=True)
                        t1_r = t1rp.tile([n, N], F32R, name="t1r", tag="t1r")
                        nc.scalar.activation(t1_r[:], p_t1[:], ACTF.Copy)
                        t1s[key] = t1_r
                    return t1s

                fuse_i = 0
                t1s = stage1(0)
                for p in range(NPLANE):
                    b, c = p // C, p % C
                    t1next = None
                    for half in range(BLK // 2):
                        s_i = p * (BLK // 2) + half
                        if s_i + R_SUP < NSUP:
                            hr_load(s_i + R_SUP, nc.sync)
                        sup = hr_t[s_i]
                        o_s = outp.tile([n, 2, N], F32, name="o", tag="o")
                        for two in range(2):
                            blk = half * 2 + two
                            bsl = bass.ts(blk, 128)
                            p_ua = ps_up.tile([n, N], F32, name="psu",
                                              tag="psu")
                            for h in range(2):
                                hs = bass.ts(h, 512)
                                nc.tensor.matmul(
                                    p_ua[:, hs], t1s["a"][:, bsl],
                                    rt_s[:, hs], start=True, stop=True)
                            q = ps_up.tile([n, N], F32, name="psu", tag="psu")
                            nc.vector.tensor_mul(
                                q[:], p_ua[:], sup[:, two, :])
                            for h in range(2):
                                hs = bass.ts(h, 512)
                                nc.tensor.matmul(
                                    q[:, hs], t1s["b"][:, bsl],
                                    rt_s[:, hs], start=False, stop=True,
                                    skip_group_check=True)
                            nc.scalar.activation(o_s[:, two, :],
                                                 q[:], ACTF.Copy)
                            fuse_i += 1
                        nc.sync.dma_start(
                            out=out_d[b, c, half * 256:(half + 1) * 256].rearrange(
                                "(two h) w -> h two w", two=2),
                            in_=o_s[:])
                        if half == 1 and p + 1 < NPLANE:
                            t1next = stage1(p + 1)
                    t1s = t1next
    nc.compile()
    return nc


_NC = None


def _get_nc():
    global _NC
    if _NC is None:
        ncb = bacc.Bacc(
            "TRN2", target_bir_lowering=False, debug=False,
            num_devices=N_CORES)
        _NC = _emit(ncb)
    return _NC


def kernel(image_lr, guide_lr, image_hr, w_box, w1, g1, b1, w2, g2, b2, w3):
    image_lr = np.ascontiguousarray(np.asarray(image_lr, np.float32))
    guide_lr = np.ascontiguousarray(np.asarray(guide_lr, np.float32))
    image_hr = np.ascontiguousarray(np.asarray(image_hr, np.float32))
    consts = _host_consts(np.asarray(w1, np.float32),
                          np.asarray(w2, np.float32),
                          np.asarray(w3, np.float32))
    gb = np.stack([np.asarray(v, np.float32) for v in (g1, b1, g2, b2)],
                  axis=1)  # [32, 4]
    nc = _get_nc()
    in_maps = []
    for i in range(N_CORES):
        sl = slice(i * BS, (i + 1) * BS)
        m = dict(xlr=image_lr[sl], ylr=guide_lr[sl], hr=image_hr[sl], gb=gb)
        m.update({k: np.ascontiguousarray(v) for k, v in consts.items()})
        in_maps.append(m)
    res = run_bass_kernel_spmd(nc, in_maps, core_ids=list(range(N_CORES)))
    global LAST_RESULT
    LAST_RESULT = res
    out = np.concatenate([res.results[i]["out"] for i in range(N_CORES)], 0)
    return out.astype(np.float32)


LAST_RESULT = None


# revision 55
# speedup vs baseline: 1.4437x; 1.2579x over previous
"""ConvGuidedFilter Trainium2 kernel (8 NeuronCores, batch-parallel).

v2 strategy (single-core timeline optimized around the 360 GB/s DMA roofline):
- Shard batch 16 -> 2 samples per core; BN batch stats AllGather'd across
  cores (local stats fail the accuracy gate).
- hr prefetch ring fills the DMA engines from t=0 while the lowres phase runs.
- Box filter = row matmul (normalization folded) + column 3-tap adds.
- 1x1-conv MLP over channel-major packed pixels; all big matmuls use fp32r
  moving tensors (bitcast, 4x faster than fp32 rows); conv2 is recomputed
  after BN2 stats instead of storing z2.
- Upsample stage2 accumulates b_up (PE, start=False) onto A_up*hr computed by
  DVE straight into PSUM; ACT copies PSUM->SBUF; SP issues paired DMAs.
"""
import os
import sys

for _p in ("/opt/trn_rl_repo", "/root/.axon_site/_ro/trn_rl_repo"):
    if os.path.isdir(_p) and _p not in sys.path:
        sys.path.insert(0, _p)

from contextlib import ExitStack

import numpy as np
import concourse.bass as bass
import concourse.tile as tile
from concourse import bacc, mybir
from concourse.bass_utils import run_bass_kernel_spmd

F32 = mybir.dt.float32
F32R = mybir.dt.float32r
BF16 = mybir.dt.bfloat16
AX = mybir.AxisListType
ALU = mybir.AluOpType
ACTF = mybir.ActivationFunctionType

B, C, n, N = 16, 3, 128, 1024  # batch, channels, lowres, hires
N_CORES, BS = 8, 2             # cores, samples per core
G = 4                          # pixel groups (32 lowres rows each)
PF = BS * 32 * n               # packed free size per partition = 8192
PT = 1024                      # pixel tile (free)
NT = PF // PT                  # 8 tiles
EPS = 1e-5
BLK = 8                        # hires row blocks per plane (1024/128)
NPLANE = BS * C                # 6 planes per core
NSUP = NPLANE * BLK // 2       # 24 paired hr/out transfers
PLANES = [(1, 0), (1, 1), (1, 2), (0, 0), (0, 1), (0, 2)]
R_SUP = 16                     # hr ring depth in supertiles (2 blocks each)


# ---------------------------------------------------------------- host consts
def _box_mats():
    Bm = np.zeros((n, n), np.float32)
    for i in range(n):
        Bm[i, max(0, i - 1):min(n, i + 2)] = 1.0
    cnt = Bm.sum(1)
    # row-box matrix with the interior normalization 1/(3*nrow) folded in;
    # edge *columns* get a 3/2 fixup after the column 3-tap sum.
    Mh = (Bm / (3.0 * cnt[:, None])).astype(np.float32)
    return np.ascontiguousarray(Mh.T)  # [h_in, h_out]


def _resize_mat():
    c = np.arange(N, dtype=np.float32) * ((n - 1) / (N - 1))
    i0 = np.clip(np.floor(c).astype(np.int64), 0, n - 2)
    t = (c - i0).astype(np.float32)
    R = np.zeros((N, n), np.float32)
    R[np.arange(N), i0] = 1.0 - t
    R[np.arange(N), i0 + 1] += t
    return np.ascontiguousarray(R.T)  # [n_in=128, n_out=1024]


def _host_consts(w1, w2, w3):
    W1b = np.zeros((64, 128), np.float32)      # [b*32 + ci*4+g, g*32+co]
    W2b = np.zeros((128, 128), np.float32)     # [g*32+ci, g*32+co]
    W3b = np.zeros((128, G * C), np.float32)   # [g*32+ci, c*4+g]
    for g in range(G):
        for ci in range(6):
            W1b[ci * G + g, g * 32:(g + 1) * 32] = w1[:, ci]
            W1b[32 + ci * G + g, g * 32:(g + 1) * 32] = w1[:, ci]
        W2b[g * 32:(g + 1) * 32, g * 32:(g + 1) * 32] = w2.T
        for c in range(C):
            W3b[g * 32:(g + 1) * 32, c * G + g] = w3[c, :]
    S32 = np.zeros((128, 32), np.float32)      # sum over groups&cores /32
    Sb = np.zeros((32, 128), np.float32)       # broadcast to groups
    for g in range(G):
        for co in range(32):
            S32[g * 32 + co, co] = 1.0 / 32.0
            Sb[co, g * 32 + co] = 1.0
    return dict(mht=_box_mats(), rt=_r32(_resize_mat()),
                w1b=_r32(W1b), w2b=_r32(W2b), w3b=_r32(W3b), s32=S32, sbc=Sb)


def _r32(x):
    # round fp32 -> fp32r-representable (bf16 hi + bf16 lo)
    import ml_dtypes
    hi = x.astype(ml_dtypes.bfloat16).astype(np.float32)
    lo = (x - hi).astype(ml_dtypes.bfloat16).astype(np.float32)
    return np.ascontiguousarray(hi + lo)


# ------------------------------------------------------------------ bass build
def _emit(nc, collectives=True, phases="AB"):
    xlr_d = nc.dram_tensor("xlr", [BS, C, n, n], F32, kind="ExternalInput")
    ylr_d = nc.dram_tensor("ylr", [BS, C, n, n], F32, kind="ExternalInput")
    hr_d = nc.dram_tensor("hr", [BS, C, N, N], F32, kind="ExternalInput")
    mht_d = nc.dram_tensor("mht", [n, n], F32, kind="ExternalInput")
    rt_d = nc.dram_tensor("rt", [n, N], F32R, kind="ExternalInput")
    w1b_d = nc.dram_tensor("w1b", [64, 128], F32R, kind="ExternalInput")
    w2b_d = nc.dram_tensor("w2b", [128, 128], F32R, kind="ExternalInput")
    w3b_d = nc.dram_tensor("w3b", [128, G * 3], F32R, kind="ExternalInput")
    s32_d = nc.dram_tensor("s32", [128, 32], F32, kind="ExternalInput")
    sbc_d = nc.dram_tensor("sbc", [32, 128], F32, kind="ExternalInput")
    gb_d = nc.dram_tensor("gb", [32, 4], F32, kind="ExternalInput")  # g1 b1 g2 b2
    out_d = nc.dram_tensor("out", [BS, C, N, N], F32, kind="ExternalOutput")

    with tile.TileContext(nc) as tc, ExitStack() as ctx:
        consts = ctx.enter_context(tc.tile_pool(name="consts", bufs=1))
        persist = ctx.enter_context(tc.tile_pool(name="persist", bufs=1))
        ringp = ctx.enter_context(tc.tile_pool(name="ring", bufs=R_SUP))
        statp = ctx.enter_context(tc.tile_pool(name="stats", bufs=1))
        dram = ctx.enter_context(tc.tile_pool(name="dram", bufs=1, space="DRAM"))

        # ---- constants into SBUF (ACT queue; all tiny except rt)
        mht_s = consts.tile([n, n], F32, name="mht", tag="mht")
        rt_s = consts.tile([n, N], F32R, name="rt", tag="rt")
        w1_s = consts.tile([64, 128], F32R, name="w1s", tag="w1s")
        w2_s = consts.tile([128, 128], F32R, name="w2s", tag="w2s")
        w3_s = consts.tile([128, G * 3], F32R, name="w3s", tag="w3s")
        s32_s = consts.tile([128, 32], F32, name="s32", tag="s32")
        sbc_s = consts.tile([32, 128], F32, name="sbc", tag="sbc")
        gb_s = consts.tile([32, 4], F32, name="gb", tag="gb")
        eps_s = consts.tile([32, 1], F32, name="eps", tag="eps")
        nc.sync.dma_start(out=mht_s[:], in_=mht_d[:])
        nc.scalar.dma_start(out=rt_s[:], in_=rt_d[:])
        nc.vector.memset(eps_s[:], EPS)
        warm_s = consts.tile([32, 1], F32, name="warm", tag="warm")
        nc.scalar.activation(warm_s[:], eps_s[:, 0:1], ACTF.Sqrt)

        # ---- lowres inputs, both samples per DMA (SP queue, first)
        x_sb = persist.tile([n, BS, C, n], F32, name="x", tag="x")
        y_sb = persist.tile([n, BS, C, n], F32, name="y", tag="y")
        nc.sync.dma_start(out=x_sb[:], in_=xlr_d.rearrange("b c h w -> h b c w"))
        nc.sync.dma_start(out=y_sb[:], in_=ylr_d.rearrange("b c h w -> h b c w"))
        for dst, srcd in ((w1_s, w1b_d), (w2_s, w2b_d), (w3_s, w3b_d),
                          (s32_s, s32_d), (sbc_s, sbc_d), (gb_s, gb_d)):
            nc.sync.dma_start(out=dst[:], in_=srcd[:])

        # ---- hr prefetch ring: 24 paired loads. First 3 + the ring-WAR
        # self-paced tail go on SP at t=0; loads 3..R_SUP-1 are issued from
        # the ACT queue at milestones inside phase A so the shared DMA-engine
        # queue stays short for latency-critical small transfers.
        hr_t = [ringp.tile([n, 2, N], BF16, name=f"hr{j}", tag="hr")
                for j in range(NSUP)]

        def hr_load(j, eng):
            # gpsimd cast-DMA fp32 -> bf16: halves SBUF so the whole hr input
            # prefetches during the lowres phase (error ~2e-3 of |out|)
            p, k = j // (BLK // 2), j % (BLK // 2)
            b, c = PLANES[p]
            nc.gpsimd.dma_start(
                out=hr_t[j][:],
                in_=hr_d[b, c, k * 256:(k + 1) * 256].rearrange(
                    "(two h) w -> h two w", two=2))

        # pace the prefetch: loads 0-2 free; loads 3..R_SUP-1 are gated on
        # phase-A milestones via 1-elem DVE copies (real data deps — the
        # scheduler reorders anything dependency-free) so the shared
        # DMA-engine queue stays short for latency-critical small transfers.
        for j in range(3):
            hr_load(j, nc.sync)

        _pace = [3, 0]

        def pace(dep_ap, only=None):
            _pace[1] += 1
            if _pace[0] < R_SUP and (only is None or _pace[1] in only):
                j = _pace[0]
                nc.vector.tensor_copy(hr_t[j][0:1, 0, 0:1], dep_ap)
                hr_load(j, nc.sync)
                _pace[0] += 1

        # persistent lowres planes
        a_sb = [persist.tile([n, C, n], F32R, name=f"a{b}", tag=f"a{b}")
                for b in range(BS)]
        bp_sb = [persist.tile([n, C, n], F32R, name=f"bp{b}", tag=f"bp{b}")
                 for b in range(BS)]
        mx_sb = [persist.tile([n, C, n], F32, name=f"mx{b}", tag=f"mx{b}")
                 for b in range(BS)]
        my_sb = [persist.tile([n, C, n], F32, name=f"my{b}", tag=f"my{b}")
                 for b in range(BS)]

        stats6 = [statp.tile([128, 2 * NT, 6], F32, name=f"st6{l}",
                             tag=f"st6{l}") for l in range(2)]
        bc_s = [statp.tile([128, 2], F32, name=f"bc{l}", tag=f"bc{l}")
                for l in range(2)]

        feat_dram = dram.tile([BS, 6, n, n], F32R, name="featd", tag="featd")
        ab_dram = dram.tile([BS, C, n, n], F32R, name="abd", tag="abd")
        ag_in = [dram.tile([32, 2], F32, name=f"agi{l}", tag=f"agi{l}")
                 for l in range(2)]
        ag_out = [dram.tile([32 * N_CORES, 2], F32, name=f"ago{l}",
                            tag=f"ago{l}") for l in range(2)]

        # ================= Phase A: lowres branch =================
        if phases == "B":
            for b in range(BS):
                nc.vector.memset(a_sb[b][:], 0.5)
                nc.vector.memset(bp_sb[b][:], 0.25)
        if phases != "B":
            with ExitStack() as actx:
                prodp = actx.enter_context(tc.tile_pool(name="prod", bufs=1))
                rbp = actx.enter_context(tc.tile_pool(name="rbp", bufs=2))
                colp = actx.enter_context(tc.tile_pool(name="colp", bufs=2))
                featp = actx.enter_context(tc.tile_pool(name="featp", bufs=1))
                mlp = actx.enter_context(tc.tile_pool(name="mlp", bufs=1))
                znp = actx.enter_context(tc.tile_pool(name="znp", bufs=2))
                tinyp = actx.enter_context(tc.tile_pool(name="tiny", bufs=2))

                ps_box_ctx = tc.tile_pool(name="ps_box", bufs=2, space="PSUM")
                ps_box = ps_box_ctx.__enter__()
                ps_z_ctx = tc.tile_pool(name="ps_z", bufs=2, space="PSUM")
                ps_z = ps_z_ctx.__enter__()
                ps_tiny_ctx = tc.tile_pool(name="ps_tiny", bufs=1, space="PSUM")
                ps_tiny = ps_tiny_ctx.__enter__()
                ps_w_ctx = tc.tile_pool(name="ps_w", bufs=1, space="PSUM")
                ps_w = ps_w_ctx.__enter__()

                def boxmean(src_ap, dst_ap, eng):
                    """dst = rowbox (PE, Mh pre-scaled) then column 3-tap sum
                    (DVE) with edge columns rescaled by 3/2 (Pool)."""
                    p_bx = ps_box.tile([n, C * n], F32, name="pbx", tag="pbx")
                    nc.tensor.matmul(
                        p_bx[:], mht_s[:],
                        src_ap.rearrange("h c w -> h (c w)"),
                        start=True, stop=True)
                    rb = rbp.tile([n, C, n], F32, name="rb", tag="rb")
                    nc.scalar.activation(
                        rb[:], p_bx[:].rearrange("h (c w) -> h c w", c=C),
                        ACTF.Copy)
                    pace(rb[0:1, 0, 0:1])
                    s1 = colp.tile([n, C, n - 2], F32, name="s1", tag="s1")
                    nc.vector.tensor_add(s1[:], rb[:, :, 0:n - 2],
                                         rb[:, :, 1:n - 1])
                    nc.vector.tensor_add(dst_ap[:, :, 1:n - 1], s1[:],
                                         rb[:, :, 2:n])
                    e0 = colp.tile([n, C, 1], F32, name="e0", tag="e0")
                    nc.gpsimd.tensor_add(e0[:], rb[:, :, 0:1], rb[:, :, 1:2])
                    nc.gpsimd.tensor_scalar_mul(dst_ap[:, :, 0:1], e0[:], 1.5)
                    e1 = colp.tile([n, C, 1], F32, name="e1", tag="e1")
                    nc.gpsimd.tensor_add(e1[:], rb[:, :, n - 2:n - 1],
                                         rb[:, :, n - 1:n])
                    nc.gpsimd.tensor_scalar_mul(dst_ap[:, :, n - 1:n],
                                                e1[:], 1.5)

                # fcb: channel-major packed feat, both samples,
                # partition q = b*24 + ci*4 + g (single-DMA transpose load).
                # Own pool, closed right after the conv2 stats pass, so
                # phase-B pools can allocate into its space early.
                fcb_ctx = tc.tile_pool(name="fcbp", bufs=1)
                fcbp = fcb_ctx.__enter__()
                fcb_s = fcbp.tile([64, 32, n], F32R, name="fcb", tag="fcb")
                z1_s = mlp.tile([128, PF], F32R, name="z1", tag="z1")
                apk_s = mlp.tile([64, 32, n], F32R, name="apk", tag="apk")

                prods = []
                for b in range(BS):
                    xy_s = prodp.tile([n, C, n], F32, name="xy", tag=f"p1{b}")
                    xx_s = prodp.tile([n, C, n], F32, name="xx", tag=f"p2{b}")
                    nc.vector.tensor_mul(xy_s[:], x_sb[:, b], y_sb[:, b])
                    nc.gpsimd.tensor_mul(xx_s[:], x_sb[:, b], x_sb[:, b])
                    prods.append((xy_s, xx_s))
                for b in range(BS):
                    feat_s = featp.tile([n, 6, n], F32R, name="feat", tag="feat")
                    xy_s, xx_s = prods[b]
                    boxmean(x_sb[:, b], mx_sb[b], nc.vector)
                    boxmean(y_sb[:, b], my_sb[b], nc.gpsimd)
                    mxy_s = prodp.tile([n, C, n], F32, name="mxy", tag=f"p1{b}")
                    mxx_s = prodp.tile([n, C, n], F32, name="mxx", tag=f"p2{b}")
                    boxmean(xy_s[:], mxy_s, nc.vector)
                    boxmean(xx_s[:], mxx_s, nc.gpsimd)
                    # cov = mxy - mx*my ; var = mxx - mx*mx  (feat = [cov, var])
                    tprod = prodp.tile([n, C, n], F32, name="tp", tag="p3")
                    nc.vector.tensor_mul(tprod[:], mx_sb[b][:], my_sb[b][:])
                    nc.vector.tensor_sub(feat_s[:, 0:3, :], mxy_s[:], tprod[:])
                    tprod2 = prodp.tile([n, C, n], F32, name="tp2", tag="p3")
                    nc.gpsimd.tensor_mul(tprod2[:], mx_sb[b][:], mx_sb[b][:])
                    nc.gpsimd.tensor_sub(feat_s[:, 3:6, :], mxx_s[:], tprod2[:])
                    nc.scalar.dma_start(
                        out=feat_dram[b].rearrange("c h w -> h c w"),
                        in_=feat_s[:])
                    nc.scalar.dma_start(
                        out=fcb_s[b * 32:b * 32 + G * 6],
                        in_=feat_dram[b].rearrange(
                            "c (g r) w -> (c g) r w", g=G))
                    pace(fcb_s[b * 32:b * 32 + 1, 0, 0:1])

                def feat_rhs(t):
                    b = t // (NT // 2)
                    half = fcb_s[b * 32:b * 32 + G * 6]
                    return half.rearrange("q r w -> q (r w)")[
                        :, bass.ts(t % (NT // 2), PT)]

                # conv1: matmul -> ACT copy to z1, DVE bn_stats on psum
                for t in range(NT):
                    p_z = ps_z.tile([128, PT], F32, name="pz", tag="pz")
                    for h in range(2):
                        hs = bass.ts(h, 512)
                        nc.tensor.matmul(
                            p_z[:, hs],
                            w1_s[t // (NT // 2) * 32:
                                 t // (NT // 2) * 32 + G * 6],
                            feat_rhs(t)[:, hs],
                            start=True, stop=True)
                    nc.scalar.activation(z1_s[:, bass.ts(t, PT)], p_z[:],
                                         ACTF.Copy)
                    if t in (1, 3, 5):
                        pace(z1_s[0:1, bass.ts(t, PT)][:, 0:1].bitcast(F32))
                    for h in range(2):
                        nc.vector.bn_stats(out=stats6[0][:, 2 * t + h, :],
                                           in_=p_z[:, bass.ts(h, 512)])

                def bn_stats_to_scalebias(l, g_col, b_col):
                    """per-partition mean/E[z^2] -> AllGather -> scale/bias."""
                    mv = tinyp.tile([128, 2], F32, name="mv", tag="mv")
                    nc.vector.bn_aggr(out=mv[:], in_=stats6[l][:])
                    mm2l = tinyp.tile([128, 1], F32, name="mm2l", tag="mm2l")
                    nc.vector.tensor_mul(mm2l[:], mv[:, 0:1], mv[:, 0:1])
                    loc2 = tinyp.tile([128, 2], F32, name="loc2", tag="loc2")
                    nc.vector.tensor_copy(loc2[:, 0:1], mv[:, 0:1])
                    nc.vector.tensor_add(loc2[:, 1:2], mv[:, 1:2], mm2l[:])
                    p_st = ps_tiny.tile([32, 2], F32, name="pst", tag="pt")
                    nc.tensor.matmul(p_st[:], s32_s[:], loc2[:],
                                     start=True, stop=True)
                    st_s = tinyp.tile([32, 2], F32, name="sts", tag="sts")
                    nc.vector.tensor_copy(st_s[:], p_st[:])
                    nc.scalar.dma_start(out=ag_in[l][:], in_=st_s[:])
                    if collectives:
                        nc.gpsimd.collective_compute(
                            "AllGather", ALU.bypass,
                            replica_groups=[list(range(N_CORES))],
                            ins=[ag_in[l][:].opt()], outs=[ag_out[l][:].opt()])
                    else:  # timing-only stand-in for the collective
                        nc.gpsimd.dma_start(
                            out=ag_out[l][0:32, :], in_=ag_in[l][:])
                    g_s = tinyp.tile([32, 2, N_CORES], F32, name="gs", tag="gs")
                    nc.scalar.dma_start(
                        out=g_s[:],
                        in_=ag_out[l][:].rearrange("(r p) s -> p s r", p=32))
                    red = tinyp.tile([32, 2], F32, name="red", tag="red")
                    nc.vector.tensor_reduce(
                        out=red[:], in_=g_s[:], axis=AX.X, op=ALU.add)
                    m_s = red[:, 0:1]
                    v_s = tinyp.tile([32, 1], F32, name="vs", tag="vs")
                    mm_s = tinyp.tile([32, 1], F32, name="mms", tag="mms")
                    sb2 = tinyp.tile([32, 2], F32, name="sb2", tag="sb2")
                    nc.vector.tensor_mul(mm_s[:], m_s, m_s)
                    nc.vector.tensor_sub(v_s[:], red[:, 1:2], mm_s[:])
                    sd_s = tinyp.tile([32, 1], F32, name="sds", tag="sds")
                    nc.scalar.activation(sd_s[:], v_s[:], ACTF.Sqrt,
                                         bias=eps_s[:])
                    nc.vector.reciprocal(sd_s[:], sd_s[:])
                    # scale = g * rinv ; bias = b - m*scale
                    nc.vector.tensor_mul(sb2[:, 0:1],
                                         gb_s[:, g_col:g_col + 1], sd_s[:])
                    nc.vector.tensor_mul(mm_s[:], m_s, sb2[:, 0:1])
                    nc.vector.tensor_sub(sb2[:, 1:2],
                                         gb_s[:, b_col:b_col + 1], mm_s[:])
                    p_bc = ps_tiny.tile([128, 2], F32, name="pbc", tag="pt")
                    nc.tensor.matmul(p_bc[:], sbc_s[:], sb2[:],
                                     start=True, stop=True)
                    nc.vector.tensor_copy(bc_s[l][:], p_bc[:])

                bn_stats_to_scalebias(0, 0, 1)

                # relu1 in place (ACT), conv2 (PE), bn_stats2 (DVE) — no z2
                for t in range(NT):
                    sl = bass.ts(t, PT)
                    nc.scalar.activation(z1_s[:, sl], z1_s[:, sl], ACTF.Relu,
                                         bias=bc_s[0][:, 1:2],
                                         scale=bc_s[0][:, 0:1])
                    p_z = ps_z.tile([128, PT], F32, name="pz", tag="pz")
                    for h in range(2):
                        hs = bass.ts(h, 512)
                        nc.tensor.matmul(p_z[:, hs], w2_s[:],
                                         z1_s[:, sl][:, hs],
                                         start=True, stop=True)
                    for h in range(2):
                        nc.vector.bn_stats(out=stats6[1][:, 2 * t + h, :],
                                           in_=p_z[:, bass.ts(h, 512)])
                    pace(stats6[1][0:1, 2 * t + 1, 0:1])

                fcb_ctx.__exit__(None, None, None)
                bn_stats_to_scalebias(1, 2, 3)

                # conv2 recompute -> fused scale/bias/relu copy -> conv3
                # -> apk (partition q = c*4+g); per-sample transpose via DRAM
                apk_f = apk_s.rearrange("q r w -> q (r w)")
                for t in list(range(NT // 2, NT)) + list(range(NT // 2)):
                    sl = bass.ts(t, PT)
                    b = t // (NT // 2)
                    p_z = ps_z.tile([128, PT], F32, name="pz", tag="pz")
                    for h in range(2):
                        hs = bass.ts(h, 512)
                        nc.tensor.matmul(p_z[:, hs], w2_s[:],
                                         z1_s[:, sl][:, hs],
                                         start=True, stop=True)
                    zn2 = znp.tile([128, PT], F32R, name="zn2", tag="zn2")
                    nc.scalar.activation(zn2[:], p_z[:], ACTF.Relu,
                                         bias=bc_s[1][:, 1:2],
                                         scale=bc_s[1][:, 0:1])
                    p_a = ps_z.tile([G * C, PT], F32, name="pa", tag="pz")
                    for h in range(2):
                        hs = bass.ts(h, 512)
                        nc.tensor.matmul(p_a[:, hs], w3_s[:],
                                         zn2[:, hs], start=True, stop=True)
                    tq = t % (NT // 2)
                    nc.vector.tensor_copy(
                        apk_f[b * 32:b * 32 + G * C, bass.ts(tq, PT)],
                        p_a[:])
                    pace(apk_f[0:1, 0:1].bitcast(F32))
                    # stream this tile's A rows out as they complete
                    nc.sync.dma_start(
                        out=ab_dram[b].rearrange(
                            "c (g rb r) w -> (c g) rb r w",
                            g=G, rb=NT // 2)[:, tq],
                        in_=apk_s[b * 32:b * 32 + G * C, bass.ts(tq, 8)])
                    if tq == NT // 2 - 1:
                        nc.sync.dma_start(
                            out=a_sb[b][:],
                            in_=ab_dram[b].rearrange("c h w -> h c w"))
                        tpb = y_sb[:, b]  # y is dead after the box phase
                        nc.vector.tensor_mul(tpb, a_sb[b][:].bitcast(F32),
                                             mx_sb[b][:])
                        nc.vector.tensor_sub(bp_sb[b][:], my_sb[b][:], tpb)

                # PE p-state warmer: dependency-free junk matmuls drain
                # whenever the PE is otherwise idle, keeping the clock ramp
                # hot through phase A's gaps (cold matmuls cost 3.7x).
                ps_w_ctx = tc.tile_pool(name="ps_w", bufs=1, space="PSUM")
                ps_w = ps_w_ctx.__enter__()
                xflat = x_sb.rearrange("h b c w -> h (b c w)")
                for _j in range(190):
                    p_w = ps_w.tile([64, 384], F32, name="pw", tag="pw")
                    nc.tensor.matmul(p_w[:], mht_s[:, 0:64], xflat[:, 0:384],
                                     start=True, stop=True)
                ps_w_ctx.__exit__(None, None, None)
                ps_tiny_ctx.__exit__(None, None, None)
                ps_z_ctx.__exit__(None, None, None)

        # ================= Phase B: upsample + fuse =================
        if phases != "A":
            with ExitStack() as uctx:
                t1rp = uctx.enter_context(tc.tile_pool(name="t1rp", bufs=4))
                outp = uctx.enter_context(tc.tile_pool(name="outp", bufs=3))
                ps_up = uctx.enter_context(
                    tc.tile_pool(name="ps_up", bufs=4, space="PSUM"))

                def stage1(p):
                    b, c = PLANES[p]
                    t1s = {}
                    for key, srcp in (("a", a_sb[b]), ("b", bp_sb[b])):
                        p_t1 = ps_up.tile([n, N], F32, name="psu", tag="psu")
                        for h in range(2):
                            hs = bass.ts(h, 512)
                            nc.tensor.matmul(p_t1[:, hs], srcp[:, c, :],
                                             rt_s[:, hs],
                                             start=True, stop=True)
                        t1_r = t1rp.tile([n, N], F32R, name="t1r", tag="t1r")
                        nc.scalar.activation(t1_r[:], p_t1[:], ACTF.Copy)
                        t1s[key] = t1_r
                    return t1s

                fuse_i = 0
                t1s = stage1(0)
                for p in range(NPLANE):
                    b, c = PLANES[p]
                    t1next = None
                    for half in range(BLK // 2):
                        s_i = p * (BLK // 2) + half
                        if s_i + R_SUP < NSUP:
                            hr_load(s_i + R_SUP, nc.sync)
                        sup = hr_t[s_i]
                        o_s = outp.tile([n, 2, N], F32, name="o", tag="o")
                        for two in range(2):
                            blk = half * 2 + two
                            bsl = bass.ts(blk, 128)
                            q = ps_up.tile([n, N], F32, name="psu", tag="psu")
                            for h in range(2):
                                hs = bass.ts(h, 512)
                                nc.tensor.matmul(
                                    q[:, hs], t1s["a"][:, bsl],
                                    rt_s[:, hs], start=True, stop=True)
                            nc.vector.tensor_mul(
                                q[:], q[:], sup[:, two, :])
                            for h in range(2):
                                hs = bass.ts(h, 512)
                                nc.tensor.matmul(
                                    q[:, hs], t1s["b"][:, bsl],
                                    rt_s[:, hs], start=False, stop=True,
                                    skip_group_check=True)
                            nc.scalar.activation(o_s[:, two, :],
                                                 q[:], ACTF.Copy)
                            fuse_i += 1
                        nc.sync.dma_start(
                            out=out_d[b, c, half * 256:(half + 1) * 256].rearrange(
                                "(two h) w -> h two w", two=2),
                            in_=o_s[:])
                        if half == 1 and p + 1 < NPLANE:
                            t1next = stage1(p + 1)
                    t1s = t1next
    nc.compile()
    return nc


_NC = None


def _get_nc():
    global _NC
    if _NC is None:
        ncb = bacc.Bacc(
            "TRN2", target_bir_lowering=False, debug=False,
            num_devices=N_CORES)
        _NC = _emit(ncb)
    return _NC


def kernel(image_lr, guide_lr, image_hr, w_box, w1, g1, b1, w2, g2, b2, w3):
    image_lr = np.ascontiguousarray(np.asarray(image_lr, np.float32))
    guide_lr = np.ascontiguousarray(np.asarray(guide_lr, np.float32))
    image_hr = np.ascontiguousarray(np.asarray(image_hr, np.float32))
    consts = _host_consts(np.asarray(w1, np.float32),
                          np.asarray(w2, np.float32),
                          np.asarray(w3, np.float32))
    gb = np.stack([np.asarray(v, np.float32) for v in (g1, b1, g2, b2)],
                  axis=1)  # [32, 4]
    nc = _get_nc()
    in_maps = []
    for i in range(N_CORES):
        sl = slice(i * BS, (i + 1) * BS)
        m = dict(xlr=image_lr[sl], ylr=guide_lr[sl], hr=image_hr[sl], gb=gb)
        m.update({k: np.ascontiguousarray(v) for k, v in consts.items()})
        in_maps.append(m)
    res = run_bass_kernel_spmd(nc, in_maps, core_ids=list(range(N_CORES)))
    global LAST_RESULT
    LAST_RESULT = res
    out = np.concatenate([res.results[i]["out"] for i in range(N_CORES)], 0)
    return out.astype(np.float32)


LAST_RESULT = None


# revision 58
# speedup vs baseline: 1.5197x; 1.0526x over previous
"""ConvGuidedFilter Trainium2 kernel (8 NeuronCores, batch-parallel).

Strategy (single-core timeline optimized around the shared DMA engines):
- Shard batch 16 -> 2 samples per core; BN batch stats AllGather'd across
  cores (local stats fail the accuracy gate).
- The full hi-res input prefetches as bf16 (gpsimd cast-DMA) into a 23-deep
  supertile ring starting at t=0; ring loads are flow-controlled by 1-elem
  copies from lowres-phase milestones so small latency-critical transfers
  are not queued behind bulk traffic.
- Box filter = row matmul (normalization folded into the matrix) + column
  3-tap adds on DVE with 3/2 edge fixup on GPSIMD.
- 1x1-conv MLP over channel-major packed pixels (4 row-groups x 32ch fill
  the 128 partitions); all big matmuls run fp32r (producers round natively);
  conv2 is recomputed after BN2 stats instead of storing z2; transposes go
  through DRAM with single-DMA rearranges (weights pre-permuted to match).
- Upsample stage2: PE writes A_up to PSUM, DVE multiplies by hr in place,
  PE accumulates b_up on top (start=False), ACT copies out, paired 2-block
  stores. A stream of tiny dependency-free matmuls keeps the PE p-state hot
  through lowres-phase gaps.
"""
import os
import sys

for _p in ("/opt/trn_rl_repo", "/root/.axon_site/_ro/trn_rl_repo"):
    if os.path.isdir(_p) and _p not in sys.path:
        sys.path.insert(0, _p)

from contextlib import ExitStack

import numpy as np
import concourse.bass as bass
import concourse.tile as tile
from concourse import bacc, mybir
from concourse.bass_utils import run_bass_kernel_spmd

F32 = mybir.dt.float32
F32R = mybir.dt.float32r
BF16 = mybir.dt.bfloat16
AX = mybir.AxisListType
ALU = mybir.AluOpType
ACTF = mybir.ActivationFunctionType

B, C, n, N = 16, 3, 128, 1024  # batch, channels, lowres, hires
N_CORES, BS = 8, 2             # cores, samples per core
G = 4                          # pixel groups (32 lowres rows each)
PF = BS * 32 * n               # packed free size per partition = 8192
PT = 1024                      # pixel tile (free)
NT = PF // PT                  # 8 tiles
EPS = 1e-5
BLK = 8                        # hires row blocks per plane (1024/128)
NPLANE = BS * C                # 6 planes per core
NSUP = NPLANE * BLK // 2       # 24 paired hr/out transfers
PLANES = [(1, 0), (1, 1), (1, 2), (0, 0), (0, 1), (0, 2)]
R_SUP = 16                     # hr ring depth in supertiles (2 blocks each)


# ---------------------------------------------------------------- host consts
def _box_mats():
    Bm = np.zeros((n, n), np.float32)
    for i in range(n):
        Bm[i, max(0, i - 1):min(n, i + 2)] = 1.0
    cnt = Bm.sum(1)
    # row-box matrix with the interior normalization 1/(3*nrow) folded in;
    # edge *columns* get a 3/2 fixup after the column 3-tap sum.
    Mh = (Bm / (3.0 * cnt[:, None])).astype(np.float32)
    return np.ascontiguousarray(Mh.T)  # [h_in, h_out]


def _resize_mat():
    c = np.arange(N, dtype=np.float32) * ((n - 1) / (N - 1))
    i0 = np.clip(np.floor(c).astype(np.int64), 0, n - 2)
    t = (c - i0).astype(np.float32)
    R = np.zeros((N, n), np.float32)
    R[np.arange(N), i0] = 1.0 - t
    R[np.arange(N), i0 + 1] += t
    return np.ascontiguousarray(R.T)  # [n_in=128, n_out=1024]


def _host_consts(w1, w2, w3):
    W1b = np.zeros((64, 128), np.float32)      # [b*32 + ci*4+g, g*32+co]
    W2b = np.zeros((128, 128), np.float32)     # [g*32+ci, g*32+co]
    W3b = np.zeros((128, G * C), np.float32)   # [g*32+ci, c*4+g]
    for g in range(G):
        for ci in range(6):
            W1b[ci * G + g, g * 32:(g + 1) * 32] = w1[:, ci]
            W1b[32 + ci * G + g, g * 32:(g + 1) * 32] = w1[:, ci]
        W2b[g * 32:(g + 1) * 32, g * 32:(g + 1) * 32] = w2.T
        for c in range(C):
            W3b[g * 32:(g + 1) * 32, c * G + g] = w3[c, :]
    S32 = np.zeros((128, 32), np.float32)      # sum over groups&cores /32
    Sb = np.zeros((32, 128), np.float32)       # broadcast to groups
    for g in range(G):
        for co in range(32):
            S32[g * 32 + co, co] = 1.0 / 32.0
            Sb[co, g * 32 + co] = 1.0
    return dict(mht=_box_mats(), rt=_r32(_resize_mat()),
                w1b=_r32(W1b), w2b=_r32(W2b), w3b=_r32(W3b), s32=S32, sbc=Sb)


def _r32(x):
    # round fp32 -> fp32r-representable (bf16 hi + bf16 lo)
    import ml_dtypes
    hi = x.astype(ml_dtypes.bfloat16).astype(np.float32)
    lo = (x - hi).astype(ml_dtypes.bfloat16).astype(np.float32)
    return np.ascontiguousarray(hi + lo)


# ------------------------------------------------------------------ bass build
def _emit(nc, collectives=True, phases="AB"):
    xlr_d = nc.dram_tensor("xlr", [BS, C, n, n], F32, kind="ExternalInput")
    ylr_d = nc.dram_tensor("ylr", [BS, C, n, n], F32, kind="ExternalInput")
    hr_d = nc.dram_tensor("hr", [BS, C, N, N], F32, kind="ExternalInput")
    mht_d = nc.dram_tensor("mht", [n, n], F32, kind="ExternalInput")
    rt_d = nc.dram_tensor("rt", [n, N], F32R, kind="ExternalInput")
    w1b_d = nc.dram_tensor("w1b", [64, 128], F32R, kind="ExternalInput")
    w2b_d = nc.dram_tensor("w2b", [128, 128], F32R, kind="ExternalInput")
    w3b_d = nc.dram_tensor("w3b", [128, G * 3], F32R, kind="ExternalInput")
    s32_d = nc.dram_tensor("s32", [128, 32], F32, kind="ExternalInput")
    sbc_d = nc.dram_tensor("sbc", [32, 128], F32, kind="ExternalInput")
    gb_d = nc.dram_tensor("gb", [32, 4], F32, kind="ExternalInput")  # g1 b1 g2 b2
    out_d = nc.dram_tensor("out", [BS, C, N, N], F32, kind="ExternalOutput")

    with tile.TileContext(nc) as tc, ExitStack() as ctx:
        consts = ctx.enter_context(tc.tile_pool(name="consts", bufs=1))
        persist = ctx.enter_context(tc.tile_pool(name="persist", bufs=1))
        ringp = ctx.enter_context(tc.tile_pool(name="ring", bufs=R_SUP))
        statp = ctx.enter_context(tc.tile_pool(name="stats", bufs=1))
        dram = ctx.enter_context(tc.tile_pool(name="dram", bufs=1, space="DRAM"))

        # ---- constants into SBUF (ACT queue; all tiny except rt)
        mht_s = consts.tile([n, n], F32, name="mht", tag="mht")
        rt_s = consts.tile([n, N], F32R, name="rt", tag="rt")
        w1_s = consts.tile([64, 128], F32R, name="w1s", tag="w1s")
        w2_s = consts.tile([128, 128], F32R, name="w2s", tag="w2s")
        w3_s = consts.tile([128, G * 3], F32R, name="w3s", tag="w3s")
        s32_s = consts.tile([128, 32], F32, name="s32", tag="s32")
        sbc_s = consts.tile([32, 128], F32, name="sbc", tag="sbc")
        gb_s = consts.tile([32, 4], F32, name="gb", tag="gb")
        eps_s = consts.tile([32, 1], F32, name="eps", tag="eps")
        nc.sync.dma_start(out=mht_s[:], in_=mht_d[:])
        nc.scalar.dma_start(out=rt_s[:], in_=rt_d[:])
        nc.vector.memset(eps_s[:], EPS)
        warm_s = consts.tile([32, 1], F32, name="warm", tag="warm")
        nc.scalar.activation(warm_s[:], eps_s[:, 0:1], ACTF.Sqrt)

        # ---- lowres inputs, both samples per DMA (SP queue, first)
        x_sb = persist.tile([n, BS, C, n], F32, name="x", tag="x")
        y_sb = persist.tile([n, BS, C, n], F32, name="y", tag="y")
        nc.sync.dma_start(out=x_sb[:], in_=xlr_d.rearrange("b c h w -> h b c w"))
        nc.sync.dma_start(out=y_sb[:], in_=ylr_d.rearrange("b c h w -> h b c w"))
        for dst, srcd in ((w1_s, w1b_d), (w2_s, w2b_d), (w3_s, w3b_d),
                          (s32_s, s32_d), (sbc_s, sbc_d), (gb_s, gb_d)):
            nc.sync.dma_start(out=dst[:], in_=srcd[:])

        # ---- hr prefetch ring: 24 paired loads. First 3 + the ring-WAR
        # self-paced tail go on SP at t=0; loads 3..R_SUP-1 are issued from
        # the ACT queue at milestones inside phase A so the shared DMA-engine
        # queue stays short for latency-critical small transfers.
        hr_t = [ringp.tile([n, 2, N], BF16, name=f"hr{j}", tag="hr")
                for j in range(NSUP)]

        def hr_load(j, eng):
            # gpsimd cast-DMA fp32 -> bf16: halves SBUF so the whole hr input
            # prefetches during the lowres phase (error ~2e-3 of |out|)
            p, k = j // (BLK // 2), j % (BLK // 2)
            b, c = PLANES[p]
            nc.gpsimd.dma_start(
                out=hr_t[j][:],
                in_=hr_d[b, c, k * 256:(k + 1) * 256].rearrange(
                    "(two h) w -> h two w", two=2))

        # pace the prefetch: loads 0-2 free; loads 3..R_SUP-1 are gated on
        # phase-A milestones via 1-elem DVE copies (real data deps — the
        # scheduler reorders anything dependency-free) so the shared
        # DMA-engine queue stays short for latency-critical small transfers.
        for j in range(3):
            hr_load(j, nc.sync)

        _pace = [3, 0]

        def pace(dep_ap, only=None):
            _pace[1] += 1
            if _pace[0] < R_SUP and (only is None or _pace[1] in only):
                j = _pace[0]
                nc.vector.tensor_copy(hr_t[j][0:1, 0, 0:1], dep_ap)
                hr_load(j, nc.sync)
                _pace[0] += 1

        # persistent lowres planes
        a_sb = [persist.tile([n, C, n], F32R, name=f"a{b}", tag=f"a{b}")
                for b in range(BS)]
        bp_sb = [persist.tile([n, C, n], F32R, name=f"bp{b}", tag=f"bp{b}")
                 for b in range(BS)]
        mx_sb = [persist.tile([n, C, n], F32, name=f"mx{b}", tag=f"mx{b}")
                 for b in range(BS)]
        my_sb = [persist.tile([n, C, n], F32, name=f"my{b}", tag=f"my{b}")
                 for b in range(BS)]

        stats6 = [statp.tile([128, 2 * NT, 6], F32, name=f"st6{l}",
                             tag=f"st6{l}") for l in range(2)]
        bc_s = [statp.tile([128, 2], F32, name=f"bc{l}", tag=f"bc{l}")
                for l in range(2)]

        feat_dram = dram.tile([BS, 6, n, n], F32R, name="featd", tag="featd")
        ab_dram = dram.tile([BS, C, n, n], F32R, name="abd", tag="abd")
        ag_in = [dram.tile([32, 2], F32, name=f"agi{l}", tag=f"agi{l}")
                 for l in range(2)]
        ag_out = [dram.tile([32 * N_CORES, 2], F32, name=f"ago{l}",
                            tag=f"ago{l}") for l in range(2)]

        # ================= Phase A: lowres branch =================
        if phases == "B":
            for b in range(BS):
                nc.vector.memset(a_sb[b][:], 0.5)
                nc.vector.memset(bp_sb[b][:], 0.25)
        if phases != "B":
            with ExitStack() as actx:
                prodp = actx.enter_context(tc.tile_pool(name="prod", bufs=1))
                rbp = actx.enter_context(tc.tile_pool(name="rbp", bufs=2))
                colp = actx.enter_context(tc.tile_pool(name="colp", bufs=2))
                featp = actx.enter_context(tc.tile_pool(name="featp", bufs=1))
                mlp = actx.enter_context(tc.tile_pool(name="mlp", bufs=1))
                znp = actx.enter_context(tc.tile_pool(name="znp", bufs=2))
                tinyp = actx.enter_context(tc.tile_pool(name="tiny", bufs=2))

                ps_box_ctx = tc.tile_pool(name="ps_box", bufs=2, space="PSUM")
                ps_box = ps_box_ctx.__enter__()
                ps_z_ctx = tc.tile_pool(name="ps_z", bufs=2, space="PSUM")
                ps_z = ps_z_ctx.__enter__()
                ps_tiny_ctx = tc.tile_pool(name="ps_tiny", bufs=1, space="PSUM")
                ps_tiny = ps_tiny_ctx.__enter__()
                ps_w_ctx = tc.tile_pool(name="ps_w", bufs=1, space="PSUM")
                ps_w = ps_w_ctx.__enter__()

                def boxmean(src_ap, dst_ap, eng):
                    """dst = rowbox (PE, Mh pre-scaled) then column 3-tap sum
                    (DVE) with edge columns rescaled by 3/2 (Pool)."""
                    p_bx = ps_box.tile([n, C * n], F32, name="pbx", tag="pbx")
                    nc.tensor.matmul(
                        p_bx[:], mht_s[:],
                        src_ap.rearrange("h c w -> h (c w)"),
                        start=True, stop=True)
                    rb = rbp.tile([n, C, n], F32, name="rb", tag="rb")
                    nc.scalar.activation(
                        rb[:], p_bx[:].rearrange("h (c w) -> h c w", c=C),
                        ACTF.Copy)
                    s1 = colp.tile([n, C, n - 2], F32, name="s1", tag="s1")
                    nc.vector.tensor_add(s1[:], rb[:, :, 0:n - 2],
                                         rb[:, :, 1:n - 1])
                    nc.vector.tensor_add(dst_ap[:, :, 1:n - 1], s1[:],
                                         rb[:, :, 2:n])
                    e0 = colp.tile([n, C, 1], F32, name="e0", tag="e0")
                    nc.gpsimd.tensor_add(e0[:], rb[:, :, 0:1], rb[:, :, 1:2])
                    nc.gpsimd.tensor_scalar_mul(dst_ap[:, :, 0:1], e0[:], 1.5)
                    e1 = colp.tile([n, C, 1], F32, name="e1", tag="e1")
                    nc.gpsimd.tensor_add(e1[:], rb[:, :, n - 2:n - 1],
                                         rb[:, :, n - 1:n])
                    nc.gpsimd.tensor_scalar_mul(dst_ap[:, :, n - 1:n],
                                                e1[:], 1.5)
                    pace(dst_ap[0:1, 0, 0:1])

                # fcb: channel-major packed feat, both samples,
                # partition q = b*24 + ci*4 + g (single-DMA transpose load).
                # Own pool, closed right after the conv2 stats pass, so
                # phase-B pools can allocate into its space early.
                fcb_ctx = tc.tile_pool(name="fcbp", bufs=1)
                fcbp = fcb_ctx.__enter__()
                fcb_s = fcbp.tile([64, 32, n], F32R, name="fcb", tag="fcb")
                z1_s = mlp.tile([128, PF], F32R, name="z1", tag="z1")
                apk_s = mlp.tile([64, 32, n], F32R, name="apk", tag="apk")

                prods = []
                for b in range(BS):
                    xy_s = prodp.tile([n, C, n], F32, name="xy", tag=f"p1{b}")
                    xx_s = prodp.tile([n, C, n], F32, name="xx", tag=f"p2{b}")
                    nc.vector.tensor_mul(xy_s[:], x_sb[:, b], y_sb[:, b])
                    nc.gpsimd.tensor_mul(xx_s[:], x_sb[:, b], x_sb[:, b])
                    prods.append((xy_s, xx_s))
                for b in range(BS):
                    feat_s = featp.tile([n, 6, n], F32R, name="feat", tag="feat")
                    xy_s, xx_s = prods[b]
                    boxmean(x_sb[:, b], mx_sb[b], nc.vector)
                    boxmean(y_sb[:, b], my_sb[b], nc.gpsimd)
                    mxy_s = prodp.tile([n, C, n], F32, name="mxy", tag=f"p1{b}")
                    mxx_s = prodp.tile([n, C, n], F32, name="mxx", tag=f"p2{b}")
                    boxmean(xy_s[:], mxy_s, nc.vector)
                    boxmean(xx_s[:], mxx_s, nc.gpsimd)
                    # cov = mxy - mx*my ; var = mxx - mx*mx  (feat = [cov, var])
                    tprod = prodp.tile([n, C, n], F32, name="tp", tag="p3")
                    nc.vector.tensor_mul(tprod[:], mx_sb[b][:], my_sb[b][:])
                    nc.vector.tensor_sub(feat_s[:, 0:3, :], mxy_s[:], tprod[:])
                    tprod2 = prodp.tile([n, C, n], F32, name="tp2", tag="p3")
                    nc.gpsimd.tensor_mul(tprod2[:], mx_sb[b][:], mx_sb[b][:])
                    nc.gpsimd.tensor_sub(feat_s[:, 3:6, :], mxx_s[:], tprod2[:])
                    nc.scalar.dma_start(
                        out=feat_dram[b].rearrange("c h w -> h c w"),
                        in_=feat_s[:])
                    nc.scalar.dma_start(
                        out=fcb_s[b * 32:b * 32 + G * 6],
                        in_=feat_dram[b].rearrange(
                            "c (g r) w -> (c g) r w", g=G))
                    pace(fcb_s[b * 32:b * 32 + 1, 0, 0:1])

                def feat_rhs(t):
                    b = t // (NT // 2)
                    half = fcb_s[b * 32:b * 32 + G * 6]
                    return half.rearrange("q r w -> q (r w)")[
                        :, bass.ts(t % (NT // 2), PT)]

                # conv1: matmul -> ACT copy to z1, DVE bn_stats on psum
                for t in range(NT):
                    p_z = ps_z.tile([128, PT], F32, name="pz", tag="pz")
                    for h in range(2):
                        hs = bass.ts(h, 512)
                        nc.tensor.matmul(
                            p_z[:, hs],
                            w1_s[t // (NT // 2) * 32:
                                 t // (NT // 2) * 32 + G * 6],
                            feat_rhs(t)[:, hs],
                            start=True, stop=True)
                    nc.scalar.activation(z1_s[:, bass.ts(t, PT)], p_z[:],
                                         ACTF.Copy)
                    if t in (1, 3, 5):
                        pace(z1_s[0:1, bass.ts(t, PT)][:, 0:1].bitcast(F32))
                    for h in range(2):
                        nc.vector.bn_stats(out=stats6[0][:, 2 * t + h, :],
                                           in_=p_z[:, bass.ts(h, 512)])

                def bn_stats_to_scalebias(l, g_col, b_col):
                    """per-partition mean/E[z^2] -> AllGather -> scale/bias."""
                    mv = tinyp.tile([128, 2], F32, name="mv", tag="mv")
                    nc.vector.bn_aggr(out=mv[:], in_=stats6[l][:])
                    mm2l = tinyp.tile([128, 1], F32, name="mm2l", tag="mm2l")
                    nc.vector.tensor_mul(mm2l[:], mv[:, 0:1], mv[:, 0:1])
                    loc2 = tinyp.tile([128, 2], F32, name="loc2", tag="loc2")
                    nc.vector.tensor_copy(loc2[:, 0:1], mv[:, 0:1])
                    nc.vector.tensor_add(loc2[:, 1:2], mv[:, 1:2], mm2l[:])
                    p_st = ps_tiny.tile([32, 2], F32, name="pst", tag="pt")
                    nc.tensor.matmul(p_st[:], s32_s[:], loc2[:],
                                     start=True, stop=True)
                    st_s = tinyp.tile([32, 2], F32, name="sts", tag="sts")
                    nc.vector.tensor_copy(st_s[:], p_st[:])
                    nc.scalar.dma_start(out=ag_in[l][:], in_=st_s[:])
                    if collectives:
                        nc.gpsimd.collective_compute(
                            "AllGather", ALU.bypass,
                            replica_groups=[list(range(N_CORES))],
                            ins=[ag_in[l][:].opt()], outs=[ag_out[l][:].opt()])
                    else:  # timing-only stand-in for the collective
                        nc.gpsimd.dma_start(
                            out=ag_out[l][0:32, :], in_=ag_in[l][:])
                    g_s = tinyp.tile([32, 2, N_CORES], F32, name="gs", tag="gs")
                    nc.scalar.dma_start(
                        out=g_s[:],
                        in_=ag_out[l][:].rearrange("(r p) s -> p s r", p=32))
                    red = tinyp.tile([32, 2], F32, name="red", tag="red")
                    nc.vector.tensor_reduce(
                        out=red[:], in_=g_s[:], axis=AX.X, op=ALU.add)
                    m_s = red[:, 0:1]
                    v_s = tinyp.tile([32, 1], F32, name="vs", tag="vs")
                    mm_s = tinyp.tile([32, 1], F32, name="mms", tag="mms")
                    sb2 = tinyp.tile([32, 2], F32, name="sb2", tag="sb2")
                    nc.vector.tensor_mul(mm_s[:], m_s, m_s)
                    nc.vector.tensor_sub(v_s[:], red[:, 1:2], mm_s[:])
                    sd_s = tinyp.tile([32, 1], F32, name="sds", tag="sds")
                    nc.scalar.activation(sd_s[:], v_s[:], ACTF.Sqrt,
                                         bias=eps_s[:])
                    nc.vector.reciprocal(sd_s[:], sd_s[:])
                    # scale = g * rinv ; bias = b - m*scale
                    nc.vector.tensor_mul(sb2[:, 0:1],
                                         gb_s[:, g_col:g_col + 1], sd_s[:])
                    nc.vector.tensor_mul(mm_s[:], m_s, sb2[:, 0:1])
                    nc.vector.tensor_sub(sb2[:, 1:2],
                                         gb_s[:, b_col:b_col + 1], mm_s[:])
                    p_bc = ps_tiny.tile([128, 2], F32, name="pbc", tag="pt")
                    nc.tensor.matmul(p_bc[:], sbc_s[:], sb2[:],
                                     start=True, stop=True)
                    nc.vector.tensor_copy(bc_s[l][:], p_bc[:])

                bn_stats_to_scalebias(0, 0, 1)

                # relu1 in place (ACT), conv2 (PE), bn_stats2 (DVE) — no z2
                for t in range(NT):
                    sl = bass.ts(t, PT)
                    nc.scalar.activation(z1_s[:, sl], z1_s[:, sl], ACTF.Relu,
                                         bias=bc_s[0][:, 1:2],
                                         scale=bc_s[0][:, 0:1])
                    p_z = ps_z.tile([128, PT], F32, name="pz", tag="pz")
                    for h in range(2):
                        hs = bass.ts(h, 512)
                        nc.tensor.matmul(p_z[:, hs], w2_s[:],
                                         z1_s[:, sl][:, hs],
                                         start=True, stop=True)
                    for h in range(2):
                        nc.vector.bn_stats(out=stats6[1][:, 2 * t + h, :],
                                           in_=p_z[:, bass.ts(h, 512)])
                    pace(stats6[1][0:1, 2 * t + 1, 0:1])

                fcb_ctx.__exit__(None, None, None)
                bn_stats_to_scalebias(1, 2, 3)

                # conv2 recompute -> fused scale/bias/relu copy -> conv3
                # -> apk (partition q = c*4+g); per-sample transpose via DRAM
                apk_f = apk_s.rearrange("q r w -> q (r w)")
                for t in list(range(NT // 2, NT)) + list(range(NT // 2)):
                    sl = bass.ts(t, PT)
                    b = t // (NT // 2)
                    p_z = ps_z.tile([128, PT], F32, name="pz", tag="pz")
                    for h in range(2):
                        hs = bass.ts(h, 512)
                        nc.tensor.matmul(p_z[:, hs], w2_s[:],
                                         z1_s[:, sl][:, hs],
                                         start=True, stop=True)
                    zn2 = znp.tile([128, PT], F32R, name="zn2", tag="zn2")
                    nc.scalar.activation(zn2[:], p_z[:], ACTF.Relu,
                                         bias=bc_s[1][:, 1:2],
                                         scale=bc_s[1][:, 0:1])
                    p_a = ps_z.tile([G * C, PT], F32, name="pa", tag="pz")
                    for h in range(2):
                        hs = bass.ts(h, 512)
                        nc.tensor.matmul(p_a[:, hs], w3_s[:],
                                         zn2[:, hs], start=True, stop=True)
                    tq = t % (NT // 2)
                    nc.vector.tensor_copy(
                        apk_f[b * 32:b * 32 + G * C, bass.ts(tq, PT)],
                        p_a[:])
                    pace(apk_f[0:1, 0:1].bitcast(F32))
                    # stream this tile's A rows out as they complete
                    nc.sync.dma_start(
                        out=ab_dram[b].rearrange(
                            "c (g rb r) w -> (c g) rb r w",
                            g=G, rb=NT // 2)[:, tq],
                        in_=apk_s[b * 32:b * 32 + G * C, bass.ts(tq, 8)])
                    if tq == NT // 2 - 1:
                        nc.sync.dma_start(
                            out=a_sb[b][:],
                            in_=ab_dram[b].rearrange("c h w -> h c w"))
                        tpb = y_sb[:, b]  # y is dead after the box phase
                        nc.vector.tensor_mul(tpb, a_sb[b][:].bitcast(F32),
                                             mx_sb[b][:])
                        nc.vector.tensor_sub(bp_sb[b][:], my_sb[b][:], tpb)

                # PE p-state warmer: dependency-free junk matmuls drain
                # whenever the PE is otherwise idle, keeping the clock ramp
                # hot through phase A's gaps (cold matmuls cost 3.7x).
                ps_w_ctx = tc.tile_pool(name="ps_w", bufs=1, space="PSUM")
                ps_w = ps_w_ctx.__enter__()
                xflat = x_sb.rearrange("h b c w -> h (b c w)")
                for _j in range(190):
                    p_w = ps_w.tile([64, 384], F32, name="pw", tag="pw")
                    nc.tensor.matmul(p_w[:], mht_s[:, 0:64], xflat[:, 0:384],
                                     start=True, stop=True)
                ps_w_ctx.__exit__(None, None, None)
                ps_tiny_ctx.__exit__(None, None, None)
                ps_z_ctx.__exit__(None, None, None)

        # ================= Phase B: upsample + fuse =================
        if phases != "A":
            with ExitStack() as uctx:
                t1rp = uctx.enter_context(tc.tile_pool(name="t1rp", bufs=4))
                outp = uctx.enter_context(tc.tile_pool(name="outp", bufs=3))
                ps_up = uctx.enter_context(
                    tc.tile_pool(name="ps_up", bufs=4, space="PSUM"))

                def stage1(p):
                    b, c = PLANES[p]
                    t1s = {}
                    for key, srcp in (("a", a_sb[b]), ("b", bp_sb[b])):
                        p_t1 = ps_up.tile([n, N], F32, name="psu", tag="psu")
                        for h in range(2):
                            hs = bass.ts(h, 512)
                            nc.tensor.matmul(p_t1[:, hs], srcp[:, c, :],
                                             rt_s[:, hs],
                                             start=True, stop=True)
                        t1_r = t1rp.tile([n, N], F32R, name="t1r", tag="t1r")
                        nc.scalar.activation(t1_r[:], p_t1[:], ACTF.Copy)
                        t1s[key] = t1_r
                    return t1s

                fuse_i = 0
                t1s = stage1(0)
                for p in range(NPLANE):
                    b, c = PLANES[p]
                    t1next = None
                    for half in range(BLK // 2):
                        s_i = p * (BLK // 2) + half
                        if s_i + R_SUP < NSUP:
                            hr_load(s_i + R_SUP, nc.sync)
                        sup = hr_t[s_i]
                        o_s = outp.tile([n, 2, N], F32, name="o", tag="o")
                        for two in range(2):
                            blk = half * 2 + two
                            bsl = bass.ts(blk, 128)
                            q = ps_up.tile([n, N], F32, name="psu", tag="psu")
                            for h in range(2):
                                hs = bass.ts(h, 512)
                                nc.tensor.matmul(
                                    q[:, hs], t1s["a"][:, bsl],
                                    rt_s[:, hs], start=True, stop=True)
                            nc.vector.tensor_mul(
                                q[:], q[:], sup[:, two, :])
                            for h in range(2):
                                hs = bass.ts(h, 512)
                                nc.tensor.matmul(
                                    q[:, hs], t1s["b"][:, bsl],
                                    rt_s[:, hs], start=False, stop=True,
                                    skip_group_check=True)
                            nc.scalar.activation(o_s[:, two, :],
                                                 q[:], ACTF.Copy)
                            fuse_i += 1
                        nc.sync.dma_start(
                            out=out_d[b, c, half * 256:(half + 1) * 256].rearrange(
                                "(two h) w -> h two w", two=2),
                            in_=o_s[:])
                        if half == 1 and p + 1 < NPLANE:
                            t1next = stage1(p + 1)
                    t1s = t1next
    nc.compile()
    return nc


_NC = None


def _get_nc():
    global _NC
    if _NC is None:
        ncb = bacc.Bacc(
            "TRN2", target_bir_lowering=False, debug=False,
            num_devices=N_CORES)
        _NC = _emit(ncb)
    return _NC


def kernel(image_lr, guide_lr, image_hr, w_box, w1, g1, b1, w2, g2, b2, w3):
    image_lr = np.ascontiguousarray(np.asarray(image_lr, np.float32))
    guide_lr = np.ascontiguousarray(np.asarray(guide_lr, np.float32))
    image_hr = np.ascontiguousarray(np.asarray(image_hr, np.float32))
    consts = _host_consts(np.asarray(w1, np.float32),
                          np.asarray(w2, np.float32),
                          np.asarray(w3, np.float32))
    gb = np.stack([np.asarray(v, np.float32) for v in (g1, b1, g2, b2)],
                  axis=1)  # [32, 4]
    nc = _get_nc()
    in_maps = []
    for i in range(N_CORES):
        sl = slice(i * BS, (i + 1) * BS)
        m = dict(xlr=image_lr[sl], ylr=guide_lr[sl], hr=image_hr[sl], gb=gb)
        m.update({k: np.ascontiguousarray(v) for k, v in consts.items()})
        in_maps.append(m)
    res = run_bass_kernel_spmd(nc, in_maps, core_ids=list(range(N_CORES)))
    global LAST_RESULT
    LAST_RESULT = res
    out = np.concatenate([res.results[i]["out"] for i in range(N_CORES)], 0)
    return out.astype(np.float32)


LAST_RESULT = None


# revision 61
# speedup vs baseline: 1.5214x; 1.0011x over previous
"""ConvGuidedFilter Trainium2 kernel (8 NeuronCores, batch-parallel).

Strategy (single-core timeline optimized around the shared DMA engines):
- Shard batch 16 -> 2 samples per core; BN batch stats AllGather'd across
  cores (local stats fail the accuracy gate).
- The full hi-res input prefetches as bf16 (gpsimd cast-DMA) into a 23-deep
  supertile ring starting at t=0; ring loads are flow-controlled by 1-elem
  copies from lowres-phase milestones so small latency-critical transfers
  are not queued behind bulk traffic.
- Box filter = row matmul (normalization folded into the matrix) + column
  3-tap adds on DVE with 3/2 edge fixup on GPSIMD.
- 1x1-conv MLP over channel-major packed pixels (4 row-groups x 32ch fill
  the 128 partitions); all big matmuls run fp32r (producers round natively);
  conv2 is recomputed after BN2 stats instead of storing z2; transposes go
  through DRAM with single-DMA rearranges (weights pre-permuted to match).
- Upsample stage2: PE writes A_up to PSUM, DVE multiplies by hr in place,
  PE accumulates b_up on top (start=False), ACT copies out, paired 2-block
  stores. A stream of tiny dependency-free matmuls keeps the PE p-state hot
  through lowres-phase gaps.
"""
import os
import sys

for _p in ("/opt/trn_rl_repo", "/root/.axon_site/_ro/trn_rl_repo"):
    if os.path.isdir(_p) and _p not in sys.path:
        sys.path.insert(0, _p)

from contextlib import ExitStack

import numpy as np
import concourse.bass as bass
import concourse.tile as tile
from concourse import bacc, mybir
from concourse.bass_utils import run_bass_kernel_spmd

F32 = mybir.dt.float32
F32R = mybir.dt.float32r
BF16 = mybir.dt.bfloat16
AX = mybir.AxisListType
ALU = mybir.AluOpType
ACTF = mybir.ActivationFunctionType

B, C, n, N = 16, 3, 128, 1024  # batch, channels, lowres, hires
N_CORES, BS = 8, 2             # cores, samples per core
G = 4                          # pixel groups (32 lowres rows each)
PF = BS * 32 * n               # packed free size per partition = 8192
PT = 1024                      # pixel tile (free)
NT = PF // PT                  # 8 tiles
EPS = 1e-5
BLK = 8                        # hires row blocks per plane (1024/128)
NPLANE = BS * C                # 6 planes per core
NSUP = NPLANE * BLK // 2       # 24 paired hr/out transfers
PLANES = [(1, 0), (1, 1), (1, 2), (0, 0), (0, 1), (0, 2)]
R_SUP = 16                     # hr ring depth in supertiles (2 blocks each)


# ---------------------------------------------------------------- host consts
def _box_mats():
    Bm = np.zeros((n, n), np.float32)
    for i in range(n):
        Bm[i, max(0, i - 1):min(n, i + 2)] = 1.0
    cnt = Bm.sum(1)
    # row-box matrix with the interior normalization 1/(3*nrow) folded in;
    # edge *columns* get a 3/2 fixup after the column 3-tap sum.
    Mh = (Bm / (3.0 * cnt[:, None])).astype(np.float32)
    return np.ascontiguousarray(Mh.T)  # [h_in, h_out]


def _resize_mat():
    c = np.arange(N, dtype=np.float32) * ((n - 1) / (N - 1))
    i0 = np.clip(np.floor(c).astype(np.int64), 0, n - 2)
    t = (c - i0).astype(np.float32)
    R = np.zeros((N, n), np.float32)
    R[np.arange(N), i0] = 1.0 - t
    R[np.arange(N), i0 + 1] += t
    return np.ascontiguousarray(R.T)  # [n_in=128, n_out=1024]


def _host_consts(w1, w2, w3):
    W1b = np.zeros((64, 128), np.float32)      # [b*32 + ci*4+g, g*32+co]
    W2b = np.zeros((128, 128), np.float32)     # [g*32+ci, g*32+co]
    W3b = np.zeros((128, G * C), np.float32)   # [g*32+ci, c*4+g]
    for g in range(G):
        for ci in range(6):
            W1b[ci * G + g, g * 32:(g + 1) * 32] = w1[:, ci]
            W1b[32 + ci * G + g, g * 32:(g + 1) * 32] = w1[:, ci]
        W2b[g * 32:(g + 1) * 32, g * 32:(g + 1) * 32] = w2.T
        for c in range(C):
            W3b[g * 32:(g + 1) * 32, c * G + g] = w3[c, :]
    S32 = np.zeros((128, 32), np.float32)      # sum over groups&cores /32
    Sb = np.zeros((32, 128), np.float32)       # broadcast to groups
    for g in range(G):
        for co in range(32):
            S32[g * 32 + co, co] = 1.0 / 32.0
            Sb[co, g * 32 + co] = 1.0
    return dict(mht=_box_mats(), rt=_r32(_resize_mat()),
                w1b=_r32(W1b), w2b=_r32(W2b), w3b=_r32(W3b), s32=S32, sbc=Sb)


def _r32(x):
    # round fp32 -> fp32r-representable (bf16 hi + bf16 lo)
    import ml_dtypes
    hi = x.astype(ml_dtypes.bfloat16).astype(np.float32)
    lo = (x - hi).astype(ml_dtypes.bfloat16).astype(np.float32)
    return np.ascontiguousarray(hi + lo)


# ------------------------------------------------------------------ bass build
def _emit(nc, collectives=True, phases="AB"):
    xlr_d = nc.dram_tensor("xlr", [BS, C, n, n], F32, kind="ExternalInput")
    ylr_d = nc.dram_tensor("ylr", [BS, C, n, n], F32, kind="ExternalInput")
    hr_d = nc.dram_tensor("hr", [BS, C, N, N], F32, kind="ExternalInput")
    mht_d = nc.dram_tensor("mht", [n, n], F32, kind="ExternalInput")
    rt_d = nc.dram_tensor("rt", [n, N], F32R, kind="ExternalInput")
    w1b_d = nc.dram_tensor("w1b", [64, 128], F32R, kind="ExternalInput")
    w2b_d = nc.dram_tensor("w2b", [128, 128], F32R, kind="ExternalInput")
    w3b_d = nc.dram_tensor("w3b", [128, G * 3], F32R, kind="ExternalInput")
    s32_d = nc.dram_tensor("s32", [128, 32], F32, kind="ExternalInput")
    sbc_d = nc.dram_tensor("sbc", [32, 128], F32, kind="ExternalInput")
    gb_d = nc.dram_tensor("gb", [32, 4], F32, kind="ExternalInput")  # g1 b1 g2 b2
    out_d = nc.dram_tensor("out", [BS, C, N, N], F32, kind="ExternalOutput")

    with tile.TileContext(nc) as tc, ExitStack() as ctx:
        consts = ctx.enter_context(tc.tile_pool(name="consts", bufs=1))
        persist = ctx.enter_context(tc.tile_pool(name="persist", bufs=1))
        ringp = ctx.enter_context(tc.tile_pool(name="ring", bufs=R_SUP))
        statp = ctx.enter_context(tc.tile_pool(name="stats", bufs=1))
        dram = ctx.enter_context(tc.tile_pool(name="dram", bufs=1, space="DRAM"))

        # ---- constants into SBUF (ACT queue; all tiny except rt)
        mht_s = consts.tile([n, n], F32, name="mht", tag="mht")
        rt_s = consts.tile([n, N], F32R, name="rt", tag="rt")
        w1_s = consts.tile([64, 128], F32R, name="w1s", tag="w1s")
        w2_s = consts.tile([128, 128], F32R, name="w2s", tag="w2s")
        w3_s = consts.tile([128, G * 3], F32R, name="w3s", tag="w3s")
        s32_s = consts.tile([128, 32], F32, name="s32", tag="s32")
        sbc_s = consts.tile([32, 128], F32, name="sbc", tag="sbc")
        gb_s = consts.tile([32, 4], F32, name="gb", tag="gb")
        eps_s = consts.tile([32, 1], F32, name="eps", tag="eps")
        nc.sync.dma_start(out=mht_s[:], in_=mht_d[:])
        nc.scalar.dma_start(out=rt_s[:], in_=rt_d[:])
        nc.vector.memset(eps_s[:], EPS)
        warm_s = consts.tile([32, 1], F32, name="warm", tag="warm")
        nc.scalar.activation(warm_s[:], eps_s[:, 0:1], ACTF.Sqrt)

        # ---- lowres inputs, both samples per DMA (SP queue, first)
        x_sb = persist.tile([n, BS, C, n], F32, name="x", tag="x")
        y_sb = persist.tile([n, BS, C, n], F32, name="y", tag="y")
        nc.sync.dma_start(out=x_sb[:], in_=xlr_d.rearrange("b c h w -> h b c w"))
        nc.sync.dma_start(out=y_sb[:], in_=ylr_d.rearrange("b c h w -> h b c w"))
        for dst, srcd in ((w1_s, w1b_d), (w2_s, w2b_d), (w3_s, w3b_d),
                          (s32_s, s32_d), (sbc_s, sbc_d), (gb_s, gb_d)):
            nc.sync.dma_start(out=dst[:], in_=srcd[:])

        # ---- hr prefetch ring: 24 paired loads. First 3 + the ring-WAR
        # self-paced tail go on SP at t=0; loads 3..R_SUP-1 are issued from
        # the ACT queue at milestones inside phase A so the shared DMA-engine
        # queue stays short for latency-critical small transfers.
        hr_t = [ringp.tile([n, 2, N], BF16, name=f"hr{j}", tag="hr")
                for j in range(NSUP)]

        def hr_load(j, eng):
            # gpsimd cast-DMA fp32 -> bf16: halves SBUF so the whole hr input
            # prefetches during the lowres phase (error ~2e-3 of |out|)
            p, k = j // (BLK // 2), j % (BLK // 2)
            b, c = PLANES[p]
            nc.gpsimd.dma_start(
                out=hr_t[j][:],
                in_=hr_d[b, c, k * 256:(k + 1) * 256].rearrange(
                    "(two h) w -> h two w", two=2))

        # pace the prefetch: loads 0-2 free; loads 3..R_SUP-1 are gated on
        # phase-A milestones via 1-elem DVE copies (real data deps — the
        # scheduler reorders anything dependency-free) so the shared
        # DMA-engine queue stays short for latency-critical small transfers.
        for j in range(3):
            hr_load(j, nc.sync)

        _pace = [3, 0]

        def pace(dep_ap, only=None):
            _pace[1] += 1
            if _pace[0] < R_SUP and (only is None or _pace[1] in only):
                j = _pace[0]
                nc.vector.tensor_copy(hr_t[j][0:1, 0, 0:1], dep_ap)
                hr_load(j, nc.sync)
                _pace[0] += 1

        # persistent lowres planes
        a_sb = [persist.tile([n, C, n], F32R, name=f"a{b}", tag=f"a{b}")
                for b in range(BS)]
        bp_sb = [persist.tile([n, C, n], F32R, name=f"bp{b}", tag=f"bp{b}")
                 for b in range(BS)]
        mx_sb = [persist.tile([n, C, n], F32, name=f"mx{b}", tag=f"mx{b}")
                 for b in range(BS)]
        my_sb = [persist.tile([n, C, n], F32, name=f"my{b}", tag=f"my{b}")
                 for b in range(BS)]

        stats6 = [statp.tile([128, 2 * NT, 6], F32, name=f"st6{l}",
                             tag=f"st6{l}") for l in range(2)]
        bc_s = [statp.tile([128, 2], F32, name=f"bc{l}", tag=f"bc{l}")
                for l in range(2)]

        feat_dram = dram.tile([BS, 6, n, n], F32R, name="featd", tag="featd")
        ab_dram = dram.tile([BS, C, n, n], F32R, name="abd", tag="abd")
        ag_in = [dram.tile([32, 2], F32, name=f"agi{l}", tag=f"agi{l}")
                 for l in range(2)]
        ag_out = [dram.tile([32 * N_CORES, 2], F32, name=f"ago{l}",
                            tag=f"ago{l}") for l in range(2)]

        # ================= Phase A: lowres branch =================
        if phases == "B":
            for b in range(BS):
                nc.vector.memset(a_sb[b][:], 0.5)
                nc.vector.memset(bp_sb[b][:], 0.25)
        if phases != "B":
            with ExitStack() as actx:
                prodp = actx.enter_context(tc.tile_pool(name="prod", bufs=1))
                rbp = actx.enter_context(tc.tile_pool(name="rbp", bufs=2))
                colp = actx.enter_context(tc.tile_pool(name="colp", bufs=2))
                featp = actx.enter_context(tc.tile_pool(name="featp", bufs=1))
                mlp = actx.enter_context(tc.tile_pool(name="mlp", bufs=1))
                znp = actx.enter_context(tc.tile_pool(name="znp", bufs=2))
                tinyp = actx.enter_context(tc.tile_pool(name="tiny", bufs=2))

                ps_box_ctx = tc.tile_pool(name="ps_box", bufs=2, space="PSUM")
                ps_box = ps_box_ctx.__enter__()
                ps_z_ctx = tc.tile_pool(name="ps_z", bufs=2, space="PSUM")
                ps_z = ps_z_ctx.__enter__()
                ps_tiny_ctx = tc.tile_pool(name="ps_tiny", bufs=1, space="PSUM")
                ps_tiny = ps_tiny_ctx.__enter__()
                ps_w_ctx = tc.tile_pool(name="ps_w", bufs=1, space="PSUM")
                ps_w = ps_w_ctx.__enter__()

                def boxmean(src_ap, dst_ap, eng):
                    """dst = rowbox (PE, Mh pre-scaled) then column 3-tap sum
                    (DVE) with edge columns rescaled by 3/2 (Pool)."""
                    p_bx = ps_box.tile([n, C * n], F32, name="pbx", tag="pbx")
                    nc.tensor.matmul(
                        p_bx[:], mht_s[:],
                        src_ap.rearrange("h c w -> h (c w)"),
                        start=True, stop=True)
                    rb = rbp.tile([n, C, n], F32, name="rb", tag="rb")
                    nc.scalar.activation(
                        rb[:], p_bx[:].rearrange("h (c w) -> h c w", c=C),
                        ACTF.Copy)
                    s1 = colp.tile([n, C, n - 2], F32, name="s1", tag="s1")
                    nc.vector.tensor_add(s1[:], rb[:, :, 0:n - 2],
                                         rb[:, :, 1:n - 1])
                    nc.vector.tensor_add(dst_ap[:, :, 1:n - 1], s1[:],
                                         rb[:, :, 2:n])
                    e0 = colp.tile([n, C, 1], F32, name="e0", tag="e0")
                    nc.gpsimd.tensor_add(e0[:], rb[:, :, 0:1], rb[:, :, 1:2])
                    nc.gpsimd.tensor_scalar_mul(dst_ap[:, :, 0:1], e0[:], 1.5)
                    e1 = colp.tile([n, C, 1], F32, name="e1", tag="e1")
                    nc.gpsimd.tensor_add(e1[:], rb[:, :, n - 2:n - 1],
                                         rb[:, :, n - 1:n])
                    nc.gpsimd.tensor_scalar_mul(dst_ap[:, :, n - 1:n],
                                                e1[:], 1.5)
                    pace(dst_ap[0:1, 0, 0:1])

                # fcb: channel-major packed feat, both samples,
                # partition q = b*24 + ci*4 + g (single-DMA transpose load).
                # Own pool, closed right after the conv2 stats pass, so
                # phase-B pools can allocate into its space early.
                fcb_ctx = tc.tile_pool(name="fcbp", bufs=1)
                fcbp = fcb_ctx.__enter__()
                fcb_s = fcbp.tile([64, 32, n], F32R, name="fcb", tag="fcb")
                z1_s = mlp.tile([128, PF], F32R, name="z1", tag="z1")
                apk_s = mlp.tile([64, 32, n], F32R, name="apk", tag="apk")

                prods = []
                for b in range(BS):
                    xy_s = prodp.tile([n, C, n], F32, name="xy", tag=f"p1{b}")
                    xx_s = prodp.tile([n, C, n], F32, name="xx", tag=f"p2{b}")
                    nc.vector.tensor_mul(xy_s[:], x_sb[:, b], y_sb[:, b])
                    nc.gpsimd.tensor_mul(xx_s[:], x_sb[:, b], x_sb[:, b])
                    prods.append((xy_s, xx_s))
                for b in range(BS):
                    feat_s = featp.tile([n, 6, n], F32R, name="feat", tag="feat")
                    xy_s, xx_s = prods[b]
                    boxmean(x_sb[:, b], mx_sb[b], nc.vector)
                    boxmean(y_sb[:, b], my_sb[b], nc.gpsimd)
                    mxy_s = prodp.tile([n, C, n], F32, name="mxy", tag=f"p1{b}")
                    mxx_s = prodp.tile([n, C, n], F32, name="mxx", tag=f"p2{b}")
                    boxmean(xy_s[:], mxy_s, nc.vector)
                    boxmean(xx_s[:], mxx_s, nc.gpsimd)
                    # cov = mxy - mx*my ; var = mxx - mx*mx  (feat = [cov, var])
                    tprod = prodp.tile([n, C, n], F32, name="tp", tag="p3")
                    nc.vector.tensor_mul(tprod[:], mx_sb[b][:], my_sb[b][:])
                    nc.vector.tensor_sub(feat_s[:, 0:3, :], mxy_s[:], tprod[:])
                    tprod2 = prodp.tile([n, C, n], F32, name="tp2", tag="p3")
                    nc.gpsimd.tensor_mul(tprod2[:], mx_sb[b][:], mx_sb[b][:])
                    nc.gpsimd.tensor_sub(feat_s[:, 3:6, :], mxx_s[:], tprod2[:])
                    nc.scalar.dma_start(
                        out=feat_dram[b].rearrange("c h w -> h c w"),
                        in_=feat_s[:])
                    nc.scalar.dma_start(
                        out=fcb_s[b * 32:b * 32 + G * 6],
                        in_=feat_dram[b].rearrange(
                            "c (g r) w -> (c g) r w", g=G))
                    pace(fcb_s[b * 32:b * 32 + 1, 0, 0:1])

                def feat_rhs(t):
                    b = t // (NT // 2)
                    half = fcb_s[b * 32:b * 32 + G * 6]
                    return half.rearrange("q r w -> q (r w)")[
                        :, bass.ts(t % (NT // 2), PT)]

                # conv1: matmul -> ACT copy to z1, DVE bn_stats on psum
                for t in range(NT):
                    p_z = ps_z.tile([128, PT], F32, name="pz", tag="pz")
                    for h in range(2):
                        hs = bass.ts(h, 512)
                        nc.tensor.matmul(
                            p_z[:, hs],
                            w1_s[t // (NT // 2) * 32:
                                 t // (NT // 2) * 32 + G * 6],
                            feat_rhs(t)[:, hs],
                            start=True, stop=True)
                    nc.scalar.activation(z1_s[:, bass.ts(t, PT)], p_z[:],
                                         ACTF.Copy)
                    if t in (1, 3, 5):
                        pace(z1_s[0:1, bass.ts(t, PT)][:, 0:1].bitcast(F32))
                    for h in range(2):
                        nc.vector.bn_stats(out=stats6[0][:, 2 * t + h, :],
                                           in_=p_z[:, bass.ts(h, 512)])

                def bn_stats_to_scalebias(l, g_col, b_col):
                    """per-partition mean/E[z^2] -> AllGather -> scale/bias."""
                    mv = tinyp.tile([128, 2], F32, name="mv", tag="mv")
                    nc.vector.bn_aggr(out=mv[:], in_=stats6[l][:])
                    mm2l = tinyp.tile([128, 1], F32, name="mm2l", tag="mm2l")
                    nc.vector.tensor_mul(mm2l[:], mv[:, 0:1], mv[:, 0:1])
                    loc2 = tinyp.tile([128, 2], F32, name="loc2", tag="loc2")
                    nc.vector.tensor_copy(loc2[:, 0:1], mv[:, 0:1])
                    nc.vector.tensor_add(loc2[:, 1:2], mv[:, 1:2], mm2l[:])
                    p_st = ps_tiny.tile([32, 2], F32, name="pst", tag="pt")
                    nc.tensor.matmul(p_st[:], s32_s[:], loc2[:],
                                     start=True, stop=True)
                    st_s = tinyp.tile([32, 2], F32, name="sts", tag="sts")
                    nc.vector.tensor_copy(st_s[:], p_st[:])
                    nc.scalar.dma_start(out=ag_in[l][:], in_=st_s[:])
                    if collectives:
                        nc.gpsimd.collective_compute(
                            "AllGather", ALU.bypass,
                            replica_groups=[list(range(N_CORES))],
                            ins=[ag_in[l][:].opt()], outs=[ag_out[l][:].opt()])
                    else:  # timing-only stand-in for the collective
                        nc.gpsimd.dma_start(
                            out=ag_out[l][0:32, :], in_=ag_in[l][:])
                    g_s = tinyp.tile([32, 2, N_CORES], F32, name="gs", tag="gs")
                    nc.scalar.dma_start(
                        out=g_s[:],
                        in_=ag_out[l][:].rearrange("(r p) s -> p s r", p=32))
                    red = tinyp.tile([32, 2], F32, name="red", tag="red")
                    nc.vector.tensor_reduce(
                        out=red[:], in_=g_s[:], axis=AX.X, op=ALU.add)
                    m_s = red[:, 0:1]
                    v_s = tinyp.tile([32, 1], F32, name="vs", tag="vs")
                    mm_s = tinyp.tile([32, 1], F32, name="mms", tag="mms")
                    sb2 = tinyp.tile([32, 2], F32, name="sb2", tag="sb2")
                    nc.vector.tensor_mul(mm_s[:], m_s, m_s)
                    nc.vector.tensor_sub(v_s[:], red[:, 1:2], mm_s[:])
                    sd_s = tinyp.tile([32, 1], F32, name="sds", tag="sds")
                    nc.scalar.activation(sd_s[:], v_s[:], ACTF.Sqrt,
                                         bias=eps_s[:])
                    nc.vector.reciprocal(sd_s[:], sd_s[:])
                    # scale = g * rinv ; bias = b - m*scale
                    nc.vector.tensor_mul(sb2[:, 0:1],
                                         gb_s[:, g_col:g_col + 1], sd_s[:])
                    nc.vector.tensor_mul(mm_s[:], m_s, sb2[:, 0:1])
                    nc.vector.tensor_sub(sb2[:, 1:2],
                                         gb_s[:, b_col:b_col + 1], mm_s[:])
                    p_bc = ps_tiny.tile([128, 2], F32, name="pbc", tag="pt")
                    nc.tensor.matmul(p_bc[:], sbc_s[:], sb2[:],
                                     start=True, stop=True)
                    nc.vector.tensor_copy(bc_s[l][:], p_bc[:])

                bn_stats_to_scalebias(0, 0, 1)

                # relu1 in place (ACT), conv2 (PE), bn_stats2 (DVE) — no z2
                for t in range(NT):
                    sl = bass.ts(t, PT)
                    nc.scalar.activation(z1_s[:, sl], z1_s[:, sl], ACTF.Relu,
                                         bias=bc_s[0][:, 1:2],
                                         scale=bc_s[0][:, 0:1])
                    p_z = ps_z.tile([128, PT], F32, name="pz", tag="pz")
                    for h in range(2):
                        hs = bass.ts(h, 512)
                        nc.tensor.matmul(p_z[:, hs], w2_s[:],
                                         z1_s[:, sl][:, hs],
                                         start=True, stop=True)
                    for h in range(2):
                        nc.vector.bn_stats(out=stats6[1][:, 2 * t + h, :],
                                           in_=p_z[:, bass.ts(h, 512)])
                    pace(stats6[1][0:1, 2 * t + 1, 0:1])

                fcb_ctx.__exit__(None, None, None)
                bn_stats_to_scalebias(1, 2, 3)

                # conv2 recompute -> fused scale/bias/relu copy -> conv3
                # -> apk (partition q = c*4+g); per-sample transpose via DRAM
                apk_f = apk_s.rearrange("q r w -> q (r w)")
                for t in list(range(NT // 2, NT)) + list(range(NT // 2)):
                    sl = bass.ts(t, PT)
                    b = t // (NT // 2)
                    p_z = ps_z.tile([128, PT], F32, name="pz", tag="pz")
                    for h in range(2):
                        hs = bass.ts(h, 512)
                        nc.tensor.matmul(p_z[:, hs], w2_s[:],
                                         z1_s[:, sl][:, hs],
                                         start=True, stop=True)
                    zn2 = znp.tile([128, PT], F32R, name="zn2", tag="zn2")
                    nc.scalar.activation(zn2[:], p_z[:], ACTF.Relu,
                                         bias=bc_s[1][:, 1:2],
                                         scale=bc_s[1][:, 0:1])
                    p_a = ps_z.tile([G * C, PT], F32, name="pa", tag="pz")
                    for h in range(2):
                        hs = bass.ts(h, 512)
                        nc.tensor.matmul(p_a[:, hs], w3_s[:],
                                         zn2[:, hs], start=True, stop=True)
                    tq = t % (NT // 2)
                    nc.vector.tensor_copy(
                        apk_f[b * 32:b * 32 + G * C, bass.ts(tq, PT)],
                        p_a[:])
                    pace(apk_f[0:1, 0:1].bitcast(F32))
                    # stream this tile's A rows out as they complete
                    nc.sync.dma_start(
                        out=ab_dram[b].rearrange(
                            "c (g rb r) w -> (c g) rb r w",
                            g=G, rb=NT // 2)[:, tq],
                        in_=apk_s[b * 32:b * 32 + G * C, bass.ts(tq, 8)])
                    if tq == NT // 2 - 1:
                        nc.sync.dma_start(
                            out=a_sb[b][:],
                            in_=ab_dram[b].rearrange("c h w -> h c w"))
                        tpb = y_sb[:, b]  # y is dead after the box phase
                        nc.vector.tensor_mul(tpb, a_sb[b][:].bitcast(F32),
                                             mx_sb[b][:])
                        nc.vector.tensor_sub(bp_sb[b][:], my_sb[b][:], tpb)

                # PE p-state warmer: dependency-free junk matmuls drain
                # whenever the PE is otherwise idle, keeping the clock ramp
                # hot through phase A's gaps (cold matmuls cost 3.7x).
                ps_w_ctx = tc.tile_pool(name="ps_w", bufs=1, space="PSUM")
                ps_w = ps_w_ctx.__enter__()
                xflat = x_sb.rearrange("h b c w -> h (b c w)")
                for _j in range(160):
                    p_w = ps_w.tile([64, 384], F32, name="pw", tag="pw")
                    nc.tensor.matmul(p_w[:], mht_s[:, 0:64], xflat[:, 0:384],
                                     start=True, stop=True)
                ps_w_ctx.__exit__(None, None, None)
                ps_tiny_ctx.__exit__(None, None, None)
                ps_z_ctx.__exit__(None, None, None)

        # ================= Phase B: upsample + fuse =================
        if phases != "A":
            with ExitStack() as uctx:
                t1rp = uctx.enter_context(tc.tile_pool(name="t1rp", bufs=4))
                outp = uctx.enter_context(tc.tile_pool(name="outp", bufs=3))
                ps_up = uctx.enter_context(
                    tc.tile_pool(name="ps_up", bufs=4, space="PSUM"))

                def stage1(p):
                    b, c = PLANES[p]
                    t1s = {}
                    for key, srcp in (("a", a_sb[b]), ("b", bp_sb[b])):
                        p_t1 = ps_up.tile([n, N], F32, name="psu", tag="psu")
                        for h in range(2):
                            hs = bass.ts(h, 512)
                            nc.tensor.matmul(p_t1[:, hs], srcp[:, c, :],
                                             rt_s[:, hs],
                                             start=True, stop=True)
                        t1_r = t1rp.tile([n, N], F32R, name="t1r", tag="t1r")
                        nc.scalar.activation(t1_r[:], p_t1[:], ACTF.Copy)
                        t1s[key] = t1_r
                    return t1s

                fuse_i = 0
                t1s = stage1(0)
                for p in range(NPLANE):
                    b, c = PLANES[p]
                    t1next = None
                    for half in range(BLK // 2):
                        s_i = p * (BLK // 2) + half
                        if s_i + R_SUP < NSUP:
                            hr_load(s_i + R_SUP, nc.sync)
                        sup = hr_t[s_i]
                        o_s = outp.tile([n, 2, N], F32, name="o", tag="o")
                        for two in range(2):
                            blk = half * 2 + two
                            bsl = bass.ts(blk, 128)
                            q = ps_up.tile([n, N], F32, name="psu", tag="psu")
                            for h in range(2):
                                hs = bass.ts(h, 512)
                                nc.tensor.matmul(
                                    q[:, hs], t1s["a"][:, bsl],
                                    rt_s[:, hs], start=True, stop=True)
                            nc.vector.tensor_mul(
                                q[:], q[:], sup[:, two, :])
                            for h in range(2):
                                hs = bass.ts(h, 512)
                                nc.tensor.matmul(
                                    q[:, hs], t1s["b"][:, bsl],
                                    rt_s[:, hs], start=False, stop=True,
                                    skip_group_check=True)
                            nc.scalar.activation(o_s[:, two, :],
                                                 q[:], ACTF.Copy)
                            fuse_i += 1
                        nc.sync.dma_start(
                            out=out_d[b, c, half * 256:(half + 1) * 256].rearrange(
                                "(two h) w -> h two w", two=2),
                            in_=o_s[:])
                        if half == 1 and p + 1 < NPLANE:
                            t1next = stage1(p + 1)
                    t1s = t1next
    nc.compile()
    return nc


_NC = None


def _get_nc():
    global _NC
    if _NC is None:
        ncb = bacc.Bacc(
            "TRN2", target_bir_lowering=False, debug=False,
            num_devices=N_CORES)
        _NC = _emit(ncb)
    return _NC


def kernel(image_lr, guide_lr, image_hr, w_box, w1, g1, b1, w2, g2, b2, w3):
    image_lr = np.ascontiguousarray(np.asarray(image_lr, np.float32))
    guide_lr = np.ascontiguousarray(np.asarray(guide_lr, np.float32))
    image_hr = np.ascontiguousarray(np.asarray(image_hr, np.float32))
    consts = _host_consts(np.asarray(w1, np.float32),
                          np.asarray(w2, np.float32),
                          np.asarray(w3, np.float32))
    gb = np.stack([np.asarray(v, np.float32) for v in (g1, b1, g2, b2)],
                  axis=1)  # [32, 4]
    nc = _get_nc()
    in_maps = []
    for i in range(N_CORES):
        sl = slice(i * BS, (i + 1) * BS)
        m = dict(xlr=image_lr[sl], ylr=guide_lr[sl], hr=image_hr[sl], gb=gb)
        m.update({k: np.ascontiguousarray(v) for k, v in consts.items()})
        in_maps.append(m)
    res = run_bass_kernel_spmd(nc, in_maps, core_ids=list(range(N_CORES)))
    global LAST_RESULT
    LAST_RESULT = res
    out = np.concatenate([res.results[i]["out"] for i in range(N_CORES)], 0)
    return out.astype(np.float32)


LAST_RESULT = None


# revision 64
# speedup vs baseline: 1.6227x; 1.0666x over previous
"""ConvGuidedFilter Trainium2 kernel (8 NeuronCores, batch-parallel).

Strategy (single-core timeline optimized around the shared DMA engines):
- Shard batch 16 -> 2 samples per core; BN batch stats AllGather'd across
  cores (local stats fail the accuracy gate).
- The full hi-res input prefetches as bf16 (gpsimd cast-DMA) into a 23-deep
  supertile ring starting at t=0; ring loads are flow-controlled by 1-elem
  copies from lowres-phase milestones so small latency-critical transfers
  are not queued behind bulk traffic.
- Box filter = row matmul (normalization folded into the matrix) + column
  3-tap adds on DVE with 3/2 edge fixup on GPSIMD.
- 1x1-conv MLP over channel-major packed pixels (4 row-groups x 32ch fill
  the 128 partitions); all big matmuls run fp32r (producers round natively);
  conv2 is recomputed after BN2 stats instead of storing z2; transposes go
  through DRAM with single-DMA rearranges (weights pre-permuted to match).
- Upsample stage2: PE writes A_up to PSUM, DVE multiplies by hr in place,
  PE accumulates b_up on top (start=False), ACT copies out, paired 2-block
  stores. A stream of tiny dependency-free matmuls keeps the PE p-state hot
  through lowres-phase gaps.
"""
import os
import sys

for _p in ("/opt/trn_rl_repo", "/root/.axon_site/_ro/trn_rl_repo"):
    if os.path.isdir(_p) and _p not in sys.path:
        sys.path.insert(0, _p)

from contextlib import ExitStack

import numpy as np
import concourse.bass as bass
import concourse.tile as tile
from concourse import bacc, mybir
from concourse.bass_utils import run_bass_kernel_spmd

F32 = mybir.dt.float32
F32R = mybir.dt.float32r
BF16 = mybir.dt.bfloat16
AX = mybir.AxisListType
ALU = mybir.AluOpType
ACTF = mybir.ActivationFunctionType

B, C, n, N = 16, 3, 128, 1024  # batch, channels, lowres, hires
N_CORES, BS = 8, 2             # cores, samples per core
G = 4                          # pixel groups (32 lowres rows each)
PF = BS * 32 * n               # packed free size per partition = 8192
PT = 1024                      # pixel tile (free)
NT = PF // PT                  # 8 tiles
EPS = 1e-5
BLK = 8                        # hires row blocks per plane (1024/128)
NPLANE = BS * C                # 6 planes per core
NSUP = NPLANE * BLK // 2       # 24 paired hr/out transfers
PLANES = [(1, 0), (1, 1), (1, 2), (0, 0), (0, 1), (0, 2)]
R_SUP = 16                     # hr ring depth in supertiles (2 blocks each)


# ---------------------------------------------------------------- host consts
def _box_mats():
    Bm = np.zeros((n, n), np.float32)
    for i in range(n):
        Bm[i, max(0, i - 1):min(n, i + 2)] = 1.0
    cnt = Bm.sum(1)
    # row-box matrix with the interior normalization 1/(3*nrow) folded in;
    # edge *columns* get a 3/2 fixup after the column 3-tap sum.
    Mh = (Bm / (3.0 * cnt[:, None])).astype(np.float32)
    return np.ascontiguousarray(Mh.T)  # [h_in, h_out]


def _resize_mat():
    c = np.arange(N, dtype=np.float32) * ((n - 1) / (N - 1))
    i0 = np.clip(np.floor(c).astype(np.int64), 0, n - 2)
    t = (c - i0).astype(np.float32)
    R = np.zeros((N, n), np.float32)
    R[np.arange(N), i0] = 1.0 - t
    R[np.arange(N), i0 + 1] += t
    return np.ascontiguousarray(R.T)  # [n_in=128, n_out=1024]


def _host_consts(w1, w2, w3):
    W1b = np.zeros((64, 128), np.float32)      # [b*32 + ci*4+g, g*32+co]
    W2b = np.zeros((128, 128), np.float32)     # [g*32+ci, g*32+co]
    W3b = np.zeros((128, G * C), np.float32)   # [g*32+ci, c*4+g]
    for g in range(G):
        for ci in range(6):
            W1b[ci * G + g, g * 32:(g + 1) * 32] = w1[:, ci]
            W1b[32 + ci * G + g, g * 32:(g + 1) * 32] = w1[:, ci]
        W2b[g * 32:(g + 1) * 32, g * 32:(g + 1) * 32] = w2.T
        for c in range(C):
            W3b[g * 32:(g + 1) * 32, c * G + g] = w3[c, :]
    S32 = np.zeros((128, 32), np.float32)      # sum over groups&cores /32
    Sb = np.zeros((32, 128), np.float32)       # broadcast to groups
    for g in range(G):
        for co in range(32):
            S32[g * 32 + co, co] = 1.0 / 32.0
            Sb[co, g * 32 + co] = 1.0
    return dict(mht=_box_mats(), rt=_r32(_resize_mat()),
                w1b=_r32(W1b), w2b=_r32(W2b), w3b=_r32(W3b), s32=S32, sbc=Sb)


def _r32(x):
    # round fp32 -> fp32r-representable (bf16 hi + bf16 lo)
    import ml_dtypes
    hi = x.astype(ml_dtypes.bfloat16).astype(np.float32)
    lo = (x - hi).astype(ml_dtypes.bfloat16).astype(np.float32)
    return np.ascontiguousarray(hi + lo)


# ------------------------------------------------------------------ bass build
def _emit(nc, collectives=True, phases="AB"):
    xlr_d = nc.dram_tensor("xlr", [BS, C, n, n], F32, kind="ExternalInput")
    ylr_d = nc.dram_tensor("ylr", [BS, C, n, n], F32, kind="ExternalInput")
    hr_d = nc.dram_tensor("hr", [BS, C, N, N], F32, kind="ExternalInput")
    mht_d = nc.dram_tensor("mht", [n, n], F32, kind="ExternalInput")
    rt_d = nc.dram_tensor("rt", [n, N], F32R, kind="ExternalInput")
    w1b_d = nc.dram_tensor("w1b", [64, 128], F32R, kind="ExternalInput")
    w2b_d = nc.dram_tensor("w2b", [128, 128], F32R, kind="ExternalInput")
    w3b_d = nc.dram_tensor("w3b", [128, G * 3], F32R, kind="ExternalInput")
    s32_d = nc.dram_tensor("s32", [128, 32], F32, kind="ExternalInput")
    sbc_d = nc.dram_tensor("sbc", [32, 128], F32, kind="ExternalInput")
    gb_d = nc.dram_tensor("gb", [32, 4], F32, kind="ExternalInput")  # g1 b1 g2 b2
    out_d = nc.dram_tensor("out", [BS, C, N, N], F32, kind="ExternalOutput")

    with tile.TileContext(nc) as tc, ExitStack() as ctx:
        consts = ctx.enter_context(tc.tile_pool(name="consts", bufs=1))
        persist = ctx.enter_context(tc.tile_pool(name="persist", bufs=1))
        ringp = ctx.enter_context(tc.tile_pool(name="ring", bufs=R_SUP))
        statp = ctx.enter_context(tc.tile_pool(name="stats", bufs=1))
        dram = ctx.enter_context(tc.tile_pool(name="dram", bufs=1, space="DRAM"))

        # ---- constants into SBUF (ACT queue; all tiny except rt)
        mht_s = consts.tile([n, n], F32, name="mht", tag="mht")
        rt_s = consts.tile([n, N], F32R, name="rt", tag="rt")
        w1_s = consts.tile([64, 128], F32R, name="w1s", tag="w1s")
        w2_s = consts.tile([128, 128], F32R, name="w2s", tag="w2s")
        w3_s = consts.tile([128, G * 3], F32R, name="w3s", tag="w3s")
        s32_s = consts.tile([128, 32], F32, name="s32", tag="s32")
        sbc_s = consts.tile([32, 128], F32, name="sbc", tag="sbc")
        gb_s = consts.tile([32, 4], F32, name="gb", tag="gb")
        eps_s = consts.tile([32, 1], F32, name="eps", tag="eps")
        nc.sync.dma_start(out=mht_s[:], in_=mht_d[:])
        nc.scalar.dma_start(out=rt_s[:], in_=rt_d[:])
        nc.vector.memset(eps_s[:], EPS)
        warm_s = consts.tile([32, 1], F32, name="warm", tag="warm")
        nc.scalar.activation(warm_s[:], eps_s[:, 0:1], ACTF.Sqrt)

        # ---- lowres inputs, both samples per DMA (SP queue, first)
        x_sb = persist.tile([n, BS, C, n], F32, name="x", tag="x")
        y_sb = persist.tile([n, BS, C, n], F32, name="y", tag="y")
        nc.sync.dma_start(out=x_sb[:], in_=xlr_d.rearrange("b c h w -> h b c w"))
        nc.sync.dma_start(out=y_sb[:], in_=ylr_d.rearrange("b c h w -> h b c w"))
        for dst, srcd in ((w1_s, w1b_d), (w2_s, w2b_d), (w3_s, w3b_d),
                          (s32_s, s32_d), (sbc_s, sbc_d), (gb_s, gb_d)):
            nc.sync.dma_start(out=dst[:], in_=srcd[:])

        # ---- hr prefetch ring: 24 paired loads. First 3 + the ring-WAR
        # self-paced tail go on SP at t=0; loads 3..R_SUP-1 are issued from
        # the ACT queue at milestones inside phase A so the shared DMA-engine
        # queue stays short for latency-critical small transfers.
        hr_t = [ringp.tile([n, 2, N], BF16, name=f"hr{j}", tag="hr")
                for j in range(NSUP)]

        def hr_load(j, eng):
            # gpsimd cast-DMA fp32 -> bf16: halves SBUF so the whole hr input
            # prefetches during the lowres phase (error ~2e-3 of |out|)
            p, k = j // (BLK // 2), j % (BLK // 2)
            b, c = PLANES[p]
            nc.gpsimd.dma_start(
                out=hr_t[j][:],
                in_=hr_d[b, c, k * 256:(k + 1) * 256].rearrange(
                    "(two h) w -> h two w", two=2))

        # pace the prefetch: loads 0-2 free; loads 3..R_SUP-1 are gated on
        # phase-A milestones via 1-elem DVE copies (real data deps — the
        # scheduler reorders anything dependency-free) so the shared
        # DMA-engine queue stays short for latency-critical small transfers.
        for j in range(3):
            hr_load(j, nc.sync)

        _pace = [3, 0]

        def pace(dep_ap, only=None):
            _pace[1] += 1
            if _pace[0] < R_SUP and (only is None or _pace[1] in only):
                j = _pace[0]
                nc.vector.tensor_copy(hr_t[j][0:1, 0, 0:1], dep_ap)
                hr_load(j, nc.sync)
                _pace[0] += 1

        # persistent lowres planes
        a_sb = [persist.tile([n, C, n], F32R, name=f"a{b}", tag=f"a{b}")
                for b in range(BS)]
        bp_sb = [persist.tile([n, C, n], F32R, name=f"bp{b}", tag=f"bp{b}")
                 for b in range(BS)]
        mx_sb = [persist.tile([n, C, n], F32, name=f"mx{b}", tag=f"mx{b}")
                 for b in range(BS)]
        my_sb = [persist.tile([n, C, n], F32, name=f"my{b}", tag=f"my{b}")
                 for b in range(BS)]

        stats6 = [statp.tile([128, 2 * NT, 6], F32, name=f"st6{l}",
                             tag=f"st6{l}") for l in range(2)]
        bc_s = [statp.tile([128, 2], F32, name=f"bc{l}", tag=f"bc{l}")
                for l in range(2)]

        feat_dram = dram.tile([BS, 6, n, n], F32R, name="featd", tag="featd")
        ab_dram = dram.tile([BS, C, n, n], F32R, name="abd", tag="abd")
        ag_in = [dram.tile([32, 2], F32, name=f"agi{l}", tag=f"agi{l}")
                 for l in range(2)]
        ag_out = [dram.tile([32 * N_CORES, 2], F32, name=f"ago{l}",
                            tag=f"ago{l}") for l in range(2)]

        # ================= Phase A: lowres branch =================
        if phases == "B":
            for b in range(BS):
                nc.vector.memset(a_sb[b][:], 0.5)
                nc.vector.memset(bp_sb[b][:], 0.25)
        if phases != "B":
            with ExitStack() as actx:
                prodp = actx.enter_context(tc.tile_pool(name="prod", bufs=1))
                rbp = actx.enter_context(tc.tile_pool(name="rbp", bufs=2))
                colp = actx.enter_context(tc.tile_pool(name="colp", bufs=2))
                featp = actx.enter_context(tc.tile_pool(name="featp", bufs=1))
                mlp = actx.enter_context(tc.tile_pool(name="mlp", bufs=1))
                znp = actx.enter_context(tc.tile_pool(name="znp", bufs=2))
                tinyp = actx.enter_context(tc.tile_pool(name="tiny", bufs=2))

                ps_box_ctx = tc.tile_pool(name="ps_box", bufs=2, space="PSUM")
                ps_box = ps_box_ctx.__enter__()
                ps_z_ctx = tc.tile_pool(name="ps_z", bufs=2, space="PSUM")
                ps_z = ps_z_ctx.__enter__()
                ps_tiny_ctx = tc.tile_pool(name="ps_tiny", bufs=1, space="PSUM")
                ps_tiny = ps_tiny_ctx.__enter__()
                ps_w_ctx = tc.tile_pool(name="ps_w", bufs=1, space="PSUM")
                ps_w = ps_w_ctx.__enter__()

                def boxmean(src_ap, dst_ap, eng):
                    """dst = rowbox (PE, Mh pre-scaled) then column 3-tap sum
                    (DVE) with edge columns rescaled by 3/2 (Pool)."""
                    p_bx = ps_box.tile([n, C * n], F32, name="pbx", tag="pbx")
                    nc.tensor.matmul(
                        p_bx[:], mht_s[:],
                        src_ap.rearrange("h c w -> h (c w)"),
                        start=True, stop=True)
                    rb = rbp.tile([n, C, n], F32, name="rb", tag="rb")
                    nc.scalar.activation(
                        rb[:], p_bx[:].rearrange("h (c w) -> h c w", c=C),
                        ACTF.Copy)
                    s1 = colp.tile([n, C, n - 2], F32, name="s1", tag="s1")
                    nc.vector.tensor_add(s1[:], rb[:, :, 0:n - 2],
                                         rb[:, :, 1:n - 1])
                    nc.vector.tensor_add(dst_ap[:, :, 1:n - 1], s1[:],
                                         rb[:, :, 2:n])
                    e0 = colp.tile([n, C, 1], F32, name="e0", tag="e0")
                    nc.gpsimd.tensor_add(e0[:], rb[:, :, 0:1], rb[:, :, 1:2])
                    nc.gpsimd.tensor_scalar_mul(dst_ap[:, :, 0:1], e0[:], 1.5)
                    e1 = colp.tile([n, C, 1], F32, name="e1", tag="e1")
                    nc.gpsimd.tensor_add(e1[:], rb[:, :, n - 2:n - 1],
                                         rb[:, :, n - 1:n])
                    nc.gpsimd.tensor_scalar_mul(dst_ap[:, :, n - 1:n],
                                                e1[:], 1.5)
                    pace(dst_ap[0:1, 0, 0:1])

                # fcb: channel-major packed feat, both samples,
                # partition q = b*24 + ci*4 + g (single-DMA transpose load).
                # Own pool, closed right after the conv2 stats pass, so
                # phase-B pools can allocate into its space early.
                fcb_ctx = tc.tile_pool(name="fcbp", bufs=1)
                fcbp = fcb_ctx.__enter__()
                fcb_s = fcbp.tile([64, 32, n], F32R, name="fcb", tag="fcb")
                z1_s = mlp.tile([128, PF], F32R, name="z1", tag="z1")
                apk_s = mlp.tile([64, 32, n], F32R, name="apk", tag="apk")

                prods = []
                for b in range(BS):
                    xy_s = prodp.tile([n, C, n], F32, name="xy", tag=f"p1{b}")
                    xx_s = prodp.tile([n, C, n], F32, name="xx", tag=f"p2{b}")
                    nc.vector.tensor_mul(xy_s[:], x_sb[:, b], y_sb[:, b])
                    nc.gpsimd.tensor_mul(xx_s[:], x_sb[:, b], x_sb[:, b])
                    prods.append((xy_s, xx_s))
                for b in range(BS):
                    feat_s = featp.tile([n, 6, n], F32R, name="feat", tag="feat")
                    xy_s, xx_s = prods[b]
                    boxmean(x_sb[:, b], mx_sb[b], nc.vector)
                    boxmean(y_sb[:, b], my_sb[b], nc.gpsimd)
                    mxy_s = prodp.tile([n, C, n], F32, name="mxy", tag=f"p1{b}")
                    mxx_s = prodp.tile([n, C, n], F32, name="mxx", tag=f"p2{b}")
                    boxmean(xy_s[:], mxy_s, nc.vector)
                    boxmean(xx_s[:], mxx_s, nc.gpsimd)
                    # cov = mxy - mx*my ; var = mxx - mx*mx  (feat = [cov, var])
                    tprod = prodp.tile([n, C, n], F32, name="tp", tag="p3")
                    nc.vector.tensor_mul(tprod[:], mx_sb[b][:], my_sb[b][:])
                    nc.vector.tensor_sub(feat_s[:, 0:3, :], mxy_s[:], tprod[:])
                    tprod2 = prodp.tile([n, C, n], F32, name="tp2", tag="p3")
                    nc.gpsimd.tensor_mul(tprod2[:], mx_sb[b][:], mx_sb[b][:])
                    nc.gpsimd.tensor_sub(feat_s[:, 3:6, :], mxx_s[:], tprod2[:])
                    nc.scalar.dma_start(
                        out=feat_dram[b].rearrange("c h w -> h c w"),
                        in_=feat_s[:])
                    nc.scalar.dma_start(
                        out=fcb_s[b * 32:b * 32 + G * 6],
                        in_=feat_dram[b].rearrange(
                            "c (g r) w -> (c g) r w", g=G))
                    pace(fcb_s[b * 32:b * 32 + 1, 0, 0:1])

                def feat_rhs(t):
                    b = t // (NT // 2)
                    half = fcb_s[b * 32:b * 32 + G * 6]
                    return half.rearrange("q r w -> q (r w)")[
                        :, bass.ts(t % (NT // 2), PT)]

                # conv1: matmul -> ACT copy to z1, DVE bn_stats on psum
                for t in range(NT):
                    p_z = ps_z.tile([128, PT], F32, name="pz", tag="pz")
                    for h in range(2):
                        hs = bass.ts(h, 512)
                        nc.tensor.matmul(
                            p_z[:, hs],
                            w1_s[t // (NT // 2) * 32:
                                 t // (NT // 2) * 32 + G * 6],
                            feat_rhs(t)[:, hs],
                            start=True, stop=True)
                    nc.scalar.activation(z1_s[:, bass.ts(t, PT)], p_z[:],
                                         ACTF.Copy)
                    if t in (1, 3, 5):
                        pace(z1_s[0:1, bass.ts(t, PT)][:, 0:1].bitcast(F32))
                    for h in range(2):
                        nc.vector.bn_stats(out=stats6[0][:, 2 * t + h, :],
                                           in_=p_z[:, bass.ts(h, 512)])

                def bn_stats_to_scalebias(l, g_col, b_col):
                    """per-partition mean/E[z^2] -> AllGather -> scale/bias."""
                    mv = tinyp.tile([128, 2], F32, name="mv", tag="mv")
                    nc.vector.bn_aggr(out=mv[:], in_=stats6[l][:])
                    mm2l = tinyp.tile([128, 1], F32, name="mm2l", tag="mm2l")
                    nc.vector.tensor_mul(mm2l[:], mv[:, 0:1], mv[:, 0:1])
                    loc2 = tinyp.tile([128, 2], F32, name="loc2", tag="loc2")
                    nc.vector.tensor_copy(loc2[:, 0:1], mv[:, 0:1])
                    nc.vector.tensor_add(loc2[:, 1:2], mv[:, 1:2], mm2l[:])
                    p_st = ps_tiny.tile([32, 2], F32, name="pst", tag="pt")
                    nc.tensor.matmul(p_st[:], s32_s[:], loc2[:],
                                     start=True, stop=True)
                    st_s = tinyp.tile([32, 2], F32, name="sts", tag="sts")
                    nc.vector.tensor_copy(st_s[:], p_st[:])
                    nc.scalar.dma_start(out=ag_in[l][:], in_=st_s[:])
                    if collectives:
                        nc.gpsimd.collective_compute(
                            "AllGather", ALU.bypass,
                            replica_groups=[list(range(N_CORES))],
                            ins=[ag_in[l][:].opt()], outs=[ag_out[l][:].opt()])
                    else:  # timing-only stand-in for the collective
                        nc.gpsimd.dma_start(
                            out=ag_out[l][0:32, :], in_=ag_in[l][:])
                    g_s = tinyp.tile([32, 2, N_CORES], F32, name="gs", tag="gs")
                    nc.scalar.dma_start(
                        out=g_s[:],
                        in_=ag_out[l][:].rearrange("(r p) s -> p s r", p=32))
                    red = tinyp.tile([32, 2], F32, name="red", tag="red")
                    nc.vector.tensor_reduce(
                        out=red[:], in_=g_s[:], axis=AX.X, op=ALU.add)
                    m_s = red[:, 0:1]
                    v_s = tinyp.tile([32, 1], F32, name="vs", tag="vs")
                    mm_s = tinyp.tile([32, 1], F32, name="mms", tag="mms")
                    sb2 = tinyp.tile([32, 2], F32, name="sb2", tag="sb2")
                    nc.vector.tensor_mul(mm_s[:], m_s, m_s)
                    nc.vector.tensor_sub(v_s[:], red[:, 1:2], mm_s[:])
                    sd_s = tinyp.tile([32, 1], F32, name="sds", tag="sds")
                    nc.scalar.activation(sd_s[:], v_s[:], ACTF.Sqrt,
                                         bias=eps_s[:])
                    nc.vector.reciprocal(sd_s[:], sd_s[:])
                    # scale = g * rinv ; bias = b - m*scale
                    nc.vector.tensor_mul(sb2[:, 0:1],
                                         gb_s[:, g_col:g_col + 1], sd_s[:])
                    nc.vector.tensor_mul(mm_s[:], m_s, sb2[:, 0:1])
                    nc.vector.tensor_sub(sb2[:, 1:2],
                                         gb_s[:, b_col:b_col + 1], mm_s[:])
                    p_bc = ps_tiny.tile([128, 2], F32, name="pbc", tag="pt")
                    nc.tensor.matmul(p_bc[:], sbc_s[:], sb2[:],
                                     start=True, stop=True)
                    nc.vector.tensor_copy(bc_s[l][:], p_bc[:])

                bn_stats_to_scalebias(0, 0, 1)

                # relu1 in place (ACT), conv2 (PE), bn_stats2 (DVE) — no z2
                for t in range(NT):
                    sl = bass.ts(t, PT)
                    nc.scalar.activation(z1_s[:, sl], z1_s[:, sl], ACTF.Relu,
                                         bias=bc_s[0][:, 1:2],
                                         scale=bc_s[0][:, 0:1])
                    p_z = ps_z.tile([128, PT], F32, name="pz", tag="pz")
                    for h in range(2):
                        hs = bass.ts(h, 512)
                        nc.tensor.matmul(p_z[:, hs], w2_s[:],
                                         z1_s[:, sl][:, hs],
                                         start=True, stop=True)
                    for h in range(2):
                        nc.vector.bn_stats(out=stats6[1][:, 2 * t + h, :],
                                           in_=p_z[:, bass.ts(h, 512)])
                    pace(stats6[1][0:1, 2 * t + 1, 0:1])

                fcb_ctx.__exit__(None, None, None)
                bn_stats_to_scalebias(1, 2, 3)

                # conv2 recompute -> fused scale/bias/relu copy -> conv3
                # -> apk (partition q = c*4+g); per-sample transpose via DRAM
                apk_f = apk_s.rearrange("q r w -> q (r w)")
                for t in list(range(NT // 2, NT)) + list(range(NT // 2)):
                    sl = bass.ts(t, PT)
                    b = t // (NT // 2)
                    p_z = ps_z.tile([128, PT], F32, name="pz", tag="pz")
                    for h in range(2):
                        hs = bass.ts(h, 512)
                        nc.tensor.matmul(p_z[:, hs], w2_s[:],
                                         z1_s[:, sl][:, hs],
                                         start=True, stop=True)
                    zn2 = znp.tile([128, PT], F32R, name="zn2", tag="zn2")
                    nc.scalar.activation(zn2[:], p_z[:], ACTF.Relu,
                                         bias=bc_s[1][:, 1:2],
                                         scale=bc_s[1][:, 0:1])
                    p_a = ps_z.tile([G * C, PT], F32, name="pa", tag="pz")
                    for h in range(2):
                        hs = bass.ts(h, 512)
                        nc.tensor.matmul(p_a[:, hs], w3_s[:],
                                         zn2[:, hs], start=True, stop=True)
                    tq = t % (NT // 2)
                    nc.vector.tensor_copy(
                        apk_f[b * 32:b * 32 + G * C, bass.ts(tq, PT)],
                        p_a[:])
                    pace(apk_f[0:1, 0:1].bitcast(F32))
                    # stream this tile's A rows out as they complete
                    nc.sync.dma_start(
                        out=ab_dram[b].rearrange(
                            "c (g rb r) w -> (c g) rb r w",
                            g=G, rb=NT // 2)[:, tq],
                        in_=apk_s[b * 32:b * 32 + G * C, bass.ts(tq, 8)])
                    if tq == NT // 2 - 1:
                        nc.sync.dma_start(
                            out=a_sb[b][:],
                            in_=ab_dram[b].rearrange("c h w -> h c w"))
                        tpb = y_sb[:, b]  # y is dead after the box phase
                        nc.vector.tensor_mul(tpb, a_sb[b][:].bitcast(F32),
                                             mx_sb[b][:])
                        nc.vector.tensor_sub(bp_sb[b][:], my_sb[b][:], tpb)

                # PE p-state warmer: dependency-free junk matmuls drain
                # whenever the PE is otherwise idle, keeping the clock ramp
                # hot through phase A's gaps (cold matmuls cost 3.7x).
                ps_w_ctx = tc.tile_pool(name="ps_w", bufs=1, space="PSUM")
                ps_w = ps_w_ctx.__enter__()
                xflat = x_sb.rearrange("h b c w -> h (b c w)")
                for _j in range(160):
                    p_w = ps_w.tile([64, 384], F32, name="pw", tag="pw")
                    nc.tensor.matmul(p_w[:], mht_s[:, 0:64], xflat[:, 0:384],
                                     start=True, stop=True)
                ps_w_ctx.__exit__(None, None, None)
                ps_tiny_ctx.__exit__(None, None, None)
                ps_z_ctx.__exit__(None, None, None)

        # ================= Phase B: upsample + fuse =================
        if phases != "A":
            with ExitStack() as uctx:
                t1rp = uctx.enter_context(tc.tile_pool(name="t1rp", bufs=4))
                outp = uctx.enter_context(tc.tile_pool(name="outp", bufs=3))
                ps_up = uctx.enter_context(
                    tc.tile_pool(name="ps_up", bufs=4, space="PSUM"))

                def stage1(p):
                    b, c = PLANES[p]
                    t1s = {}
                    for key, srcp in (("a", a_sb[b]), ("b", bp_sb[b])):
                        p_t1 = ps_up.tile([n, N], F32, name="psu", tag="psu")
                        for h in range(2):
                            hs = bass.ts(h, 512)
                            nc.tensor.matmul(p_t1[:, hs], srcp[:, c, :],
                                             rt_s[:, hs],
                                             start=True, stop=True)
                        t1_r = t1rp.tile([n, N], F32R, name="t1r", tag="t1r")
                        nc.scalar.activation(t1_r[:], p_t1[:], ACTF.Copy)
                        t1s[key] = t1_r
                    return t1s

                fuse_i = 0
                t1s = stage1(0)
                for p in range(NPLANE):
                    b, c = PLANES[p]
                    t1next = None
                    for half in range(BLK // 2):
                        s_i = p * (BLK // 2) + half
                        if s_i + R_SUP < NSUP:
                            hr_load(s_i + R_SUP, nc.sync)
                        sup = hr_t[s_i]
                        o_s = outp.tile([n, 2, N], F32, name="o", tag="o")
                        for two in range(2):
                            blk = half * 2 + two
                            bsl = bass.ts(blk, 128)
                            q = ps_up.tile([n, N], F32, name="psu", tag="psu")
                            for h in range(2):
                                hs = bass.ts(h, 512)
                                nc.tensor.matmul(
                                    q[:, hs], t1s["a"][:, bsl],
                                    rt_s[:, hs], start=True, stop=True)
                            nc.vector.tensor_mul(
                                q[:], q[:], sup[:, two, :])
                            for h in range(2):
                                hs = bass.ts(h, 512)
                                nc.tensor.matmul(
                                    q[:, hs], t1s["b"][:, bsl],
                                    rt_s[:, hs], start=False, stop=True,
                                    skip_group_check=True)
                            nc.scalar.activation(o_s[:, two, :],
                                                 q[:], ACTF.Copy)
                            fuse_i += 1
                        nc.sync.dma_start(
                            out=out_d[b, c, half * 256:(half + 1) * 256].rearrange(
                                "(two h) w -> h two w", two=2),
                            in_=o_s[:])
                        if half == 1 and p + 1 < NPLANE:
                            t1next = stage1(p + 1)
                    t1s = t1next
    nc.compile()
    return nc


_NC = None


def _get_nc():
    global _NC
    if _NC is None:
        ncb = bacc.Bacc(
            "TRN2", target_bir_lowering=False, debug=False,
            num_devices=N_CORES)
        _NC = _emit(ncb)
    return _NC


def kernel(image_lr, guide_lr, image_hr, w_box, w1, g1, b1, w2, g2, b2, w3):
    image_lr = np.ascontiguousarray(np.asarray(image_lr, np.float32))
    guide_lr = np.ascontiguousarray(np.asarray(guide_lr, np.float32))
    image_hr = np.ascontiguousarray(np.asarray(image_hr, np.float32))
    consts = _host_consts(np.asarray(w1, np.float32),
                          np.asarray(w2, np.float32),
                          np.asarray(w3, np.float32))
    gb = np.stack([np.asarray(v, np.float32) for v in (g1, b1, g2, b2)],
                  axis=1)  # [32, 4]
    nc = _get_nc()
    in_maps = []
    for i in range(N_CORES):
        sl = slice(i * BS, (i + 1) * BS)
        m = dict(xlr=image_lr[sl], ylr=guide_lr[sl], hr=image_hr[sl], gb=gb)
        m.update({k: np.ascontiguousarray(v) for k, v in consts.items()})
        in_maps.append(m)
    res = run_bass_kernel_spmd(nc, in_maps, core_ids=list(range(N_CORES)))
    global LAST_RESULT
    LAST_RESULT = res
    out = np.concatenate([res.results[i]["out"] for i in range(N_CORES)], 0)
    return out.astype(np.float32)


LAST_RESULT = None
